# revision 11
# baseline (speedup 1.0000x reference)
"""MiniMax-M2 decoder layer on 8 trn2 NeuronCores.

Sharding: sequence-sharded attention (each core owns 512 tokens of the
flattened (B*S)=4096 token stream and recomputes the 512-token KV halo
locally -> no collectives in the attention block), tensor-parallel MLP
(IM=8192 sharded 1024/core; AllGather of the ln2-normed activations,
ReduceScatter of the w2 partial sums in bf16).

v2 restructure vs baseline:
- no DRAM bf16 pre-cast phase: weights stream via gpsimd casting DMAs
  (f32 DRAM -> bf16 SBUF in flight), in big [128,512]+ tiles.
- w1/w3 SBUF-resident bf16, loaded once during the attention phase.
- ln1_w/ln2_w folded into wq/wk/wv/w1/w3 host-side.
- AllGather split 4-ways along own-token columns, outputs in Shared
  DRAM; MLP chunks start as soon as the first slice lands.
- MLP keeps all gate activations (gs) in SBUF; w2 runs hs-slice-major
  with a pipelined 4-way ReduceScatter + per-slice residual tail.
- x2 residual stream in bf16 (rel-err budget allows).

Self-contained: includes the BIR wait-splitting fix this container's
walrus build needs (1 semaphore wait per instruction max).
"""

import json
import sys
import types

import numpy as np

import concourse.bass as bass
import concourse.mybir as mybir
import concourse.tile as tile
from concourse.masks import make_identity

# ---------------------------------------------------------------- constants
B, S, HID = 2, 2048, 2048
H, HK, D = 16, 4, 128
RD = 64
IM = 8192
WIN = 512
EPS = 1e-6
THETA = 10000.0
SCALE = D ** -0.5

NCORES = 8
TOK = 512              # own tokens per core
EXT = 1024             # halo + own
IMC = IM // NCORES     # 1024 im rows per core
NEG = -1e9

F32 = mybir.dt.float32
F32R = mybir.dt.float32r
BF16 = mybir.dt.bfloat16
AF = mybir.ActivationFunctionType

KT = 8                 # 128-wide key tiles over EXT
NM = HID // 128        # 16 hid tiles
NMI = IMC // 128       # 8 im tiles per core
NQ = 4                 # AllGather token-column split
QW = TOK // NQ         # 128 columns per AG slice

# ------------------------------------------------------- walrus wait-split fix
MAX_WAITS = 1


def _split_excess_waits(bir_bytes: bytes) -> bytes:
    m = json.loads(bir_bytes)
    ctr = [0]

    def fix_insts(insts):
        out = []
        for ins in insts:
            si = ins.get("sync_info")
            ow = (si or {}).get("on_wait") or []
            if len(ow) > MAX_WAITS:
                eng = ins["engine"]
                keep = ow[-MAX_WAITS:]
                excess = ow[:-MAX_WAITS]
                ins["sync_info"]["on_wait"] = keep
                for i in range(0, len(excess), MAX_WAITS):
                    ctr[0] += 1
                    out.append({
                        "debug": ins.get("debug", 0),
                        "engine": eng,
                        "ins": [],
                        "name": f"I-waitfix-{ctr[0]}",
                        "opcode": "NoOp",
                        "outs": [],
                        "sync_info": {"on_update": [],
                                      "on_wait": excess[i:i + MAX_WAITS]},
                        "text_hint": "waitfix",
                    })
            out.append(ins)
        return out

    def walk(o):
        if isinstance(o, dict):
            if isinstance(o.get("instructions"), list):
                o["instructions"] = fix_insts(o["instructions"])
            for v in o.values():
                walk(v)
        elif isinstance(o, list):
            for v in o:
                walk(v)

    walk(m)
    return json.dumps(m).encode()


class _BassFixed(bass.Bass):
    def to_json_bytes(self) -> bytes:
        return _split_excess_waits(super().to_json_bytes())


def _register_ntff_hook():
    """Provide antenv.axon_hooks (missing in this image) so trace=True works."""
    if "antenv.axon_hooks" in sys.modules:
        return
    try:
        import trn_agent_boot.trn_boot as tb
    except ImportError:
        return
    mod = types.ModuleType("antenv.axon_hooks")
    holder = [None]
    mod.set_axon_ntff_profile_hook = lambda h: holder.__setitem__(0, h)
    mod.get_axon_ntff_profile_hook = lambda: holder[0]
    sys.modules["antenv.axon_hooks"] = mod
    try:
        mod.set_axon_ntff_profile_hook(
            tb._ntff_profile_via_ctypes("/opt/axon/libaxon_pjrt.so"))
    except Exception:
        pass


# ---------------------------------------------------------------- the program
def build_nc():
    nc = _BassFixed(num_devices=NCORES, target_bir_lowering=False)

    xT = nc.dram_tensor("xT", [HID, EXT], F32R, kind="ExternalInput")
    wqT = nc.dram_tensor("wqT", [HID, H * D], F32R, kind="ExternalInput")
    wkT = nc.dram_tensor("wkT", [HID, HK * D], F32R, kind="ExternalInput")
    wvT = nc.dram_tensor("wvT", [HID, HK * D], F32R, kind="ExternalInput")
    woT = nc.dram_tensor("woT", [H * D, HID], F32R, kind="ExternalInput")
    w1T = nc.dram_tensor("w1T", [HID, IMC], F32R, kind="ExternalInput")
    w3T = nc.dram_tensor("w3T", [HID, IMC], F32R, kind="ExternalInput")
    w2T = nc.dram_tensor("w2T", [IMC, HID], F32R, kind="ExternalInput")
    qnw = nc.dram_tensor("qnw", [128, 16], F32, kind="ExternalInput")
    knw = nc.dram_tensor("knw", [128, 4], F32, kind="ExternalInput")
    cos_q = nc.dram_tensor("cos_q", [RD, TOK], F32, kind="ExternalInput")
    sinS_q = nc.dram_tensor("sinS_q", [RD, TOK], F32, kind="ExternalInput")
    cos_k = nc.dram_tensor("cos_k", [RD, EXT], F32, kind="ExternalInput")
    sinS_k = nc.dram_tensor("sinS_k", [RD, EXT], F32, kind="ExternalInput")
    halo = nc.dram_tensor("halo", [128, 8], F32, kind="ExternalInput")
    band = nc.dram_tensor("band", [128, 1408], BF16, kind="ExternalInput")
    ones_r = nc.dram_tensor("ones_r", [1, 128], F32R, kind="ExternalInput")
    ones_b = nc.dram_tensor("ones_b", [128, 1], BF16, kind="ExternalInput")

    out = nc.dram_tensor("out", [TOK, HID], F32, kind="ExternalOutput")

    with tile.TileContext(nc) as tc:
        with tc.tile_pool(name="consts", bufs=1) as cst, \
             tc.tile_pool(name="res", bufs=1) as res, \
             tc.tile_pool(name="dram", bufs=1, space="DRAM") as dram, \
             tc.tile_pool(name="rowps", bufs=3, space="PSUM") as rps, \
             tc.tile_pool(name="bcps", bufs=1, space="PSUM") as bcps:

            # ---------------- constants
            oner = cst.tile([1, 128], F32R)
            nc.sync.dma_start(oner[:], ones_r[:])
            oneb = cst.tile([128, 1], BF16)
            nc.sync.dma_start(oneb[:], ones_b[:])
            qnw_s = cst.tile([128, 16], F32)
            nc.sync.dma_start(qnw_s[:], qnw[:])
            knw_s = cst.tile([128, 4], F32)
            nc.sync.dma_start(knw_s[:], knw[:])
            halo_s = cst.tile([128, 8], F32)
            nc.sync.dma_start(halo_s[:], halo[:])
            band_s = cst.tile([128, 1408], BF16)
            nc.sync.dma_start(band_s[:], band[:])
            eps_s = cst.tile([1, 1], F32)
            nc.vector.memset(eps_s[:], EPS)
            identF = cst.tile([128, 128], F32)
            make_identity(nc, identF[:])
            identB = cst.tile([128, 128], BF16)
            nc.vector.tensor_copy(identB[:], identF[:])

            # tiny warm-up collective: absorbs the one-time CC entry
            # barrier (~40us) under the projection phase
            dummy_i = dram.tile([128, 16], BF16, name="dummy_i")
            dummy_o = dram.tile([NCORES, 128, 16], BF16,
                                addr_space="Shared", name="dummy_o")
            dmv = cst.tile([128, 16], BF16)
            nc.vector.memset(dmv[:], 0.0)
            nc.sync.dma_start(dummy_i[:], dmv[:])
            nc.gpsimd.collective_compute(
                "AllGather", mybir.AluOpType.bypass,
                replica_groups=[list(range(NCORES))],
                ins=[dummy_i.opt()], outs=[dummy_o.opt()],
            )

            # resident MLP weights (loads issued after QKV weight streaming)
            w1s = res.tile([128, NM, IMC], BF16)
            w3s = res.tile([128, NM, IMC], BF16)

            # internal DRAM
            ag_in = [dram.tile([HID, QW], BF16, name=f"ag_in{q}")
                     for q in range(NQ)]
            ag_out = [dram.tile([NCORES, HID, QW], BF16, addr_space="Shared",
                                name=f"ag_out{q}") for q in range(NQ)]
            rs_in = [dram.tile([NCORES * TOK, 512], BF16, name=f"rs_in{h}")
                     for h in range(4)]
            rs_out = [dram.tile([TOK, 512], BF16, name=f"rs_out{h}")
                      for h in range(4)]
            x2tok = dram.tile([TOK, HID], BF16)

            # ======== attention block scope
            with tc.tile_pool(name="qkv", bufs=1) as qkv, \
                 tc.tile_pool(name="smalls", bufs=1) as sml:
                qT = qkv.tile([128, H, TOK], BF16)
                kT = qkv.tile([128, HK, EXT], BF16)
                Vb = qkv.tile([128, KT, HK * D], BF16)

                def bcast_row(row_f32r, width, tag):
                    """[1,width] F32R row -> list of (psum[128,w], j, w)."""
                    parts = []
                    for j in range(0, width, 512):
                        w = min(512, width - j)
                        p = bcps.tile([128, w], F32, tag="bc",
                                      name=f"bc_{tag}_{j}")
                        nc.tensor.matmul(p[:], oner[:],
                                         row_f32r[:, j:j + w],
                                         start=True, stop=True)
                        parts.append((p, j, w))
                    return parts

                def row_rsqrt_bcast(acc_parts, width, denom, tag):
                    """psum [1,*] sum-of-squares parts -> [128,*] psum
                    chunks with 1/sqrt(acc/denom + eps) bcast."""
                    srow = sml.tile([1, width], F32, tag="srow")
                    for (p, j, w) in acc_parts:
                        nc.scalar.activation(out=srow[:, j:j + w], in_=p[:],
                                             func=AF.Sqrt, bias=eps_s[:],
                                             scale=1.0 / denom)
                    rrow = sml.tile([1, width], F32R, tag="rrow")
                    with nc.allow_low_precision(reason="f32r intended"):
                        nc.vector.reciprocal(rrow[:], srow[:])
                    return bcast_row(rrow, width, tag)

                # ============ projections, two 512-token halves (own first)
                with tc.tile_pool(name="hp", bufs=1) as hp, \
                     tc.tile_pool(name="nrm", bufs=1) as nrm, \
                     tc.tile_pool(name="xs", bufs=3) as xs, \
                     tc.tile_pool(name="sqp", bufs=3) as sqp, \
                     tc.tile_pool(name="ws", bufs=3) as ws, \
                     tc.tile_pool(name="accP", bufs=4, space="PSUM") as accP:
                    # rope tables
                    cq_s = nrm.tile([RD, TOK], F32)
                    nc.sync.dma_start(cq_s[:], cos_q[:])
                    sq_s = nrm.tile([RD, TOK], F32)
                    nc.sync.dma_start(sq_s[:], sinS_q[:])
                    ck_s = nrm.tile([RD, EXT], F32)
                    nc.sync.dma_start(ck_s[:], cos_k[:])
                    sk_s = nrm.tile([RD, EXT], F32)
                    nc.sync.dma_start(sk_s[:], sinS_k[:])

                    def rope(t3, nh, width, cos_t, sinS_t):
                        c3 = cos_t[:].rearrange(
                            "p (g t) -> p g t",
                            g=1).broadcast_to([RD, nh, width])
                        s3 = sinS_t[:].rearrange(
                            "p (g t) -> p g t",
                            g=1).broadcast_to([RD, nh, width])
                        # partner halves moved onto matching partitions
                        qsw = nrm.tile([RD, nh, width], BF16, tag="rsw")
                        nc.sync.dma_start(qsw[0:32], t3[32:64])
                        nc.sync.dma_start(qsw[32:64], t3[0:32])
                        t1 = nrm.tile([RD, nh, width], BF16, tag="rt1")
                        nc.vector.tensor_mul(t1[:], t3[0:RD], c3)
                        nc.vector.tensor_mul(qsw[:], qsw[:], s3)
                        nc.vector.tensor_add(t3[0:RD], t1[:], qsw[:])

                    vT = hp.tile([128, HK, EXT], BF16, tag="vT")
                    for half in (1, 0):
                        c0 = half * 512
                        acc = rps.tile([1, 512], F32, tag="row")
                        for i in range(NM):
                            xt = xs.tile([128, 512], F32R, tag="xt")
                            nc.sync.dma_start(
                                xt[:], xT[i * 128:(i + 1) * 128, c0:c0 + 512])
                            sq = sqp.tile([128, 512], BF16, tag="sq")
                            nc.vector.tensor_mul(sq[:], xt[:], xt[:])
                            nc.tensor.matmul(acc[:], oneb[:], sq[:],
                                             start=(i == 0), stop=(i == NM - 1))
                        s1b = row_rsqrt_bcast([(acc, 0, 512)], 512, HID,
                                               "l1")[0][0]
                        hTh = hp.tile([128, NM, 512], BF16, tag="h")
                        for i in range(NM):
                            xt = xs.tile([128, 512], F32R, tag="xt")
                            nc.sync.dma_start(
                                xt[:], xT[i * 128:(i + 1) * 128, c0:c0 + 512])
                            nc.vector.tensor_mul(hTh[:, i, :], xt[:], s1b[:])

                        if half == 1:
                            # Q projection (own tokens only), 4-head groups
                            for mg in range(4):
                                pq = [accP.tile([128, 512], F32, tag="acc",
                                                 name=f"pq{j}")
                                      for j in range(4)]
                                for i in range(NM):
                                    wq_t = ws.tile([128, 512], BF16, tag="wq")
                                    nc.gpsimd.dma_start(
                                        wq_t[:],
                                        wqT[i * 128:(i + 1) * 128,
                                            mg * 512:(mg + 1) * 512])
                                    for j in range(4):
                                        nc.tensor.matmul(
                                            pq[j][:],
                                            wq_t[:, j * 128:(j + 1) * 128],
                                            hTh[:, i, :],
                                            start=(i == 0), stop=(i == NM - 1))
                                for j in range(4):
                                    nc.vector.tensor_copy(
                                        qT[:, mg * 4 + j, :], pq[j][:])

                        # K / V for this half (i-outer, 4 kv heads inner)
                        pk = [accP.tile([128, 512], F32, tag="acc",
                                         name=f"pk{g}")
                              for g in range(4)]
                        for i in range(NM):
                            wk_t = ws.tile([128, 512], BF16, tag="wk")
                            nc.gpsimd.dma_start(
                                wk_t[:], wkT[i * 128:(i + 1) * 128, :])
                            for g in range(HK):
                                nc.tensor.matmul(
                                    pk[g][:], wk_t[:, g * 128:(g + 1) * 128],
                                    hTh[:, i, :],
                                    start=(i == 0), stop=(i == NM - 1))
                        for g in range(HK):
                            nc.vector.tensor_copy(kT[:, g, c0:c0 + 512],
                                                  pk[g][:])
                        pv = [accP.tile([128, 512], F32, tag="acc",
                                         name=f"pv{g}")
                              for g in range(4)]
                        for i in range(NM):
                            wv_t = ws.tile([128, 512], BF16, tag="wv")
                            nc.gpsimd.dma_start(
                                wv_t[:], wvT[i * 128:(i + 1) * 128, :])
                            for g in range(HK):
                                nc.tensor.matmul(
                                    pv[g][:], wv_t[:, g * 128:(g + 1) * 128],
                                    hTh[:, i, :],
                                    start=(i == 0), stop=(i == NM - 1))
                        for g in range(HK):
                            nc.vector.tensor_copy(vT[:, g, c0:c0 + 512],
                                                  pv[g][:])

                        if half == 1:
                            # fused q RMSNorm + rope; overlaps half-0 K/V
                            accq = rps.tile([1, 512], F32, tag="row")
                            sqq = nrm.tile([128, TOK], BF16, tag="nsq")
                            for h in range(H):
                                nc.vector.tensor_mul(sqq[:], qT[:, h, :],
                                                     qT[:, h, :])
                                nc.tensor.matmul(accq[:], oneb[:], sqq[:],
                                                 start=(h == 0),
                                                 stop=(h == H - 1))
                            cqb = row_rsqrt_bcast([(accq, 0, 512)], TOK,
                                                  H * D, "qn")[0][0]
                            cqb3 = cqb[:].rearrange(
                                "p (g t) -> p g t",
                                g=1).broadcast_to([128, H, TOK])
                            nc.vector.tensor_mul(qT[:], qT[:], cqb3)
                            for h in range(H):
                                nc.vector.tensor_scalar_mul(
                                    qT[:, h, :], qT[:, h, :],
                                    qnw_s[:, h:h + 1])
                            rope(qT[:, 0:8, :], 8, TOK, cq_s, sq_s)
                            rope(qT[:, 8:16, :], 8, TOK, cq_s, sq_s)

                    # resident MLP weight loads (gpsimd queue, after qkv w)
                    for i in range(NM):
                        nc.gpsimd.dma_start(
                            w1s[:, i, :], w1T[i * 128:(i + 1) * 128, :])
                    for i in range(NM):
                        nc.gpsimd.dma_start(
                            w3s[:, i, :], w3T[i * 128:(i + 1) * 128, :])

                    # k RMSNorm + rope
                    acck_lo = rps.tile([1, 512], F32, tag="row")
                    acck_hi = rps.tile([1, 512], F32, tag="row")
                    sqk = nrm.tile([128, EXT], BF16, tag="nsqk")
                    for g in range(HK):
                        nc.vector.tensor_mul(sqk[:], kT[:, g, :], kT[:, g, :])
                        nc.tensor.matmul(acck_lo[:], oneb[:], sqk[:, 0:512],
                                         start=(g == 0), stop=(g == HK - 1))
                        nc.tensor.matmul(acck_hi[:], oneb[:], sqk[:, 512:1024],
                                         start=(g == 0), stop=(g == HK - 1))
                    ckb_parts = row_rsqrt_bcast(
                        [(acck_lo, 0, 512), (acck_hi, 512, 512)],
                        EXT, HK * D, "kn")
                    for (pck, j, w) in ckb_parts:
                        v = pck[:].rearrange(
                            "p (g t) -> p g t",
                            g=1).broadcast_to([128, HK, w])
                        nc.vector.tensor_mul(kT[:, :, j:j + w],
                                             kT[:, :, j:j + w], v)
                    for g in range(HK):
                        nc.vector.tensor_scalar_mul(
                            kT[:, g, :], kT[:, g, :], knw_s[:, g:g + 1])
                    rope(kT[:], HK, EXT, ck_s, sk_s)

                    # transpose vT -> token-major bf16 Vb
                    for kt in range(KT):
                        for g in range(HK):
                            pt = accP.tile([128, 128], BF16, tag="acc")
                            nc.tensor.transpose(
                                pt[:], vT[:, g, kt * 128:(kt + 1) * 128],
                                identB[:])
                            nc.vector.tensor_copy(
                                Vb[:, kt, g * 128:(g + 1) * 128], pt[:])

                # ============ sliding-window attention
                with tc.tile_pool(name="attn", bufs=1) as ap, \
                     tc.tile_pool(name="es", bufs=12) as es, \
                     tc.tile_pool(name="bps", bufs=4, space="PSUM") as bps:
                    attnT = ap.tile([128, H, TOK], BF16)
                    for h in range(H):
                        g = h // (H // HK)
                        exps = []
                        for kt in range(KT):
                            ps = bps.tile([128, 512], F32, tag="big")
                            nc.tensor.matmul(
                                ps[:], kT[:, g, kt * 128:(kt + 1) * 128],
                                qT[:, h, :], start=True, stop=True)
                            e = es.tile([128, 512], BF16, tag="e")
                            nc.scalar.activation(
                                out=e[:], in_=ps[:], func=AF.Exp,
                                bias=halo_s[:, kt:kt + 1], scale=SCALE)
                            nc.vector.tensor_mul(
                                e[:], e[:],
                                band_s[:, 896 - 128 * kt:1408 - 128 * kt])
                            exps.append(e)
                        den = rps.tile([1, 512], F32, tag="row")
                        for kt in range(KT):
                            nc.tensor.matmul(den[:], oneb[:], exps[kt][:],
                                             start=(kt == 0),
                                             stop=(kt == KT - 1))
                        dr = sml.tile([1, 512], F32, tag="dr")
                        nc.vector.tensor_copy(dr[:], den[:])
                        drr = sml.tile([1, 512], F32R, tag="drr")
                        with nc.allow_low_precision(reason="f32r intended"):
                            nc.vector.reciprocal(drr[:], dr[:])
                        rb = bcast_row(drr, 512, "rden")[0][0]
                        rbs = sml.tile([128, 512], F32R, tag="rbs")
                        nc.vector.tensor_copy(rbs[:], rb[:])
                        po = bps.tile([128, 512], F32, tag="big")
                        for kt in range(KT):
                            nc.tensor.matmul(
                                po[:],
                                Vb[:, kt, g * 128:(g + 1) * 128],
                                exps[kt][:], start=(kt == 0),
                                stop=(kt == KT - 1))
                        nc.vector.tensor_mul(attnT[:, h, :], po[:], rbs[:])

                    # ============ o_proj + residual + ln2
                    with tc.tile_pool(name="x2", bufs=1) as x2p, \
                         tc.tile_pool(name="xs2", bufs=3) as xs2:
                        x2T = x2p.tile([128, NM, TOK], BF16)
                        acc2 = rps.tile([1, 512], F32, tag="row")
                        for mg in range(4):
                            px = [bps.tile([128, 512], F32, tag="big",
                                            name=f"px{j}")
                                  for j in range(4)]
                            for i in range(NM):
                                wo_t = xs2.tile([128, 512], BF16, tag="wo")
                                nc.gpsimd.dma_start(
                                    wo_t[:], woT[i * 128:(i + 1) * 128,
                                                 mg * 512:(mg + 1) * 512])
                                for j in range(4):
                                    nc.tensor.matmul(
                                        px[j][:],
                                        wo_t[:, j * 128:(j + 1) * 128],
                                        attnT[:, i, :],
                                        start=(i == 0), stop=(i == NM - 1))
                            for j in range(4):
                                m = mg * 4 + j
                                xo = xs2.tile([128, TOK], F32R, tag="xo")
                                nc.sync.dma_start(
                                    xo[:], xT[m * 128:(m + 1) * 128, 512:1024])
                                nc.vector.tensor_add(x2T[:, m, :], px[j][:],
                                                     xo[:])
                                sq2 = xs2.tile([128, TOK], BF16, tag="sq2")
                                nc.vector.tensor_mul(sq2[:], x2T[:, m, :],
                                                     x2T[:, m, :])
                                nc.tensor.matmul(acc2[:], oneb[:], sq2[:],
                                                 start=(m == 0),
                                                 stop=(m == NM - 1))

                        # ln2 -> h2 -> AG inputs (first, so AG starts early)
                        s2b = row_rsqrt_bcast([(acc2, 0, 512)], TOK, HID,
                                                "l2")[0][0]
                        for m in range(NM):
                            h2t = xs2.tile([128, TOK], BF16, tag="h2t")
                            nc.vector.tensor_mul(h2t[:], x2T[:, m, :], s2b[:])
                            for q in range(NQ):
                                nc.sync.dma_start(
                                    ag_in[q][m * 128:(m + 1) * 128, :],
                                    h2t[:, q * QW:(q + 1) * QW])

                        for q in range(NQ):
                            nc.gpsimd.collective_compute(
                                "AllGather", mybir.AluOpType.bypass,
                                replica_groups=[list(range(NCORES))],
                                ins=[ag_in[q].opt()], outs=[ag_out[q].opt()],
                            )

                        # x2 token-major -> DRAM (overlaps the AllGather)
                        for tt in range(4):
                            for grp in range(4):
                                ts = xs2.tile([128, 512], BF16, tag="x2t")
                                for j in range(4):
                                    m = grp * 4 + j
                                    pt = bps.tile([128, 128], BF16, tag="big")
                                    nc.tensor.transpose(
                                        pt[:],
                                        x2T[:, m, tt * 128:(tt + 1) * 128],
                                        identB[:])
                                    nc.vector.tensor_copy(
                                        ts[:, j * 128:(j + 1) * 128], pt[:])
                                nc.sync.dma_start(
                                    x2tok[tt * 128:(tt + 1) * 128,
                                          grp * 512:(grp + 1) * 512], ts[:])

            # ============ TP MLP: chunks of 512 tokens (4 cores x 128 cols)
            with tc.tile_pool(name="gsp", bufs=1) as gsp, \
                 tc.tile_pool(name="mh", bufs=2) as mh, \
                 tc.tile_pool(name="silp", bufs=2) as silp, \
                 tc.tile_pool(name="w2s", bufs=2) as w2sp, \
                 tc.tile_pool(name="po", bufs=2) as pop, \
                 tc.tile_pool(name="mps", bufs=4, space="PSUM") as mps:
                gs = gsp.tile([128, NMI, NCORES * TOK], BF16)
                for q in range(NQ):
                    for quad in range(2):
                        h2c = mh.tile([128, NM, 512], BF16, tag="h2")
                        for i in range(NM):
                            src = ag_out[q][quad * 4:(quad + 1) * 4,
                                            i * 128:(i + 1) * 128, :]
                            nc.sync.dma_start(
                                h2c[:, i, :].rearrange(
                                    "p (c w) -> p c w", c=4),
                                src.rearrange("c p w -> p c w"))
                        for m in range(NMI):
                            a = mps.tile([128, 512], F32, tag="m")
                            for i in range(NM):
                                nc.tensor.matmul(
                                    a[:], w1s[:, i, m * 128:(m + 1) * 128],
                                    h2c[:, i, :],
                                    start=(i == 0), stop=(i == NM - 1))
                            sil = silp.tile([128, 512], BF16, tag="sil")
                            nc.scalar.activation(out=sil[:], in_=a[:],
                                                 func=AF.Silu)
                            b = mps.tile([128, 512], F32, tag="m")
                            for i in range(NM):
                                nc.tensor.matmul(
                                    b[:], w3s[:, i, m * 128:(m + 1) * 128],
                                    h2c[:, i, :],
                                    start=(i == 0), stop=(i == NM - 1))
                            dst = gs[:, m,
                                     quad * 4 * TOK:(quad * 4 + 4) * TOK]
                            dst3 = dst.rearrange("p (c w) -> p c w", c=4)
                            nc.vector.tensor_mul(
                                dst3[:, :, q * QW:(q + 1) * QW],
                                sil[:].rearrange("p (c w) -> p c w", c=4),
                                b[:].rearrange("p (c w) -> p c w", c=4))

                # w2 by output-column slice; pipelined ReduceScatter + tail
                for hs in range(4):
                    w2c = w2sp.tile([128, NMI, 512], BF16, tag="w2c")
                    for m in range(NMI):
                        nc.gpsimd.dma_start(
                            w2c[:, m, :], w2T[m * 128:(m + 1) * 128,
                                              hs * 512:(hs + 1) * 512])
                    for tt in range(NCORES * TOK // 128):
                        op_ = mps.tile([128, 512], F32, tag="m")
                        for m in range(NMI):
                            nc.tensor.matmul(
                                op_[:], gs[:, m, tt * 128:(tt + 1) * 128],
                                w2c[:, m, :],
                                start=(m == 0), stop=(m == NMI - 1))
                        pb = pop.tile([128, 512], BF16, tag="pb")
                        nc.vector.tensor_copy(pb[:], op_[:])
                        nc.sync.dma_start(
                            rs_in[hs][tt * 128:(tt + 1) * 128, :], pb[:])
                    nc.gpsimd.collective_compute(
                        "ReduceScatter", mybir.AluOpType.add,
                        replica_groups=[list(range(NCORES))],
                        ins=[rs_in[hs].opt()], outs=[rs_out[hs].opt()],
                    )

                with tc.tile_pool(name="tail", bufs=4) as tp:
                    for hs in range(4):
                        for tt in range(4):
                            rsb = tp.tile([128, 512], BF16, tag="rsb")
                            nc.scalar.dma_start(
                                rsb[:], rs_out[hs][tt * 128:(tt + 1) * 128, :])
                            x2s = tp.tile([128, 512], BF16, tag="x2s")
                            nc.scalar.dma_start(
                                x2s[:], x2tok[tt * 128:(tt + 1) * 128,
                                              hs * 512:(hs + 1) * 512])
                            os_ = tp.tile([128, 512], F32, tag="os")
                            nc.vector.tensor_add(os_[:], rsb[:], x2s[:])
                            nc.scalar.dma_start(
                                out[tt * 128:(tt + 1) * 128,
                                    hs * 512:(hs + 1) * 512], os_[:])

    return nc


# ---------------------------------------------------------------- host side
def _rope_tables(pos):
    inv = 1.0 / (THETA ** (np.arange(0, RD, 2, dtype=np.float32) / RD))
    f = pos[:, None].astype(np.float32) * inv[None, :]
    emb = np.concatenate([f, f], axis=-1)          # [T, RD]
    cos = np.ascontiguousarray(np.cos(emb).T)      # [RD, T]
    sin = np.sin(emb).T
    sinS = sin.copy()
    sinS[0:32] = -sin[0:32]
    return cos.astype(np.float32), np.ascontiguousarray(sinS).astype(np.float32)


def _band_mask():
    import ml_dtypes
    p = np.arange(128)[:, None]
    u = np.arange(1408)[None, :]
    m = ((u >= p + 384) & (u <= p + 896)).astype(np.float32)
    return m.astype(ml_dtypes.bfloat16)


def _prepare_in_maps(hidden_states, wq, wk, wv, wo, q_norm_w, k_norm_w,
                     ln1_w, ln2_w, w1, w2, w3):
    import ml_dtypes
    xf = np.ascontiguousarray(hidden_states.reshape(B * S, HID))
    # fold ln1_w into wq/wk/wv rows, ln2_w into w1/w3 rows
    wqTn = np.ascontiguousarray(wq.T * ln1_w[:, None])
    wkTn = np.ascontiguousarray(wk.T * ln1_w[:, None])
    wvTn = np.ascontiguousarray(wv.T * ln1_w[:, None])
    woTn = np.ascontiguousarray(wo.T)
    w1Tn = np.ascontiguousarray(w1.T * ln2_w[:, None])
    w3Tn = np.ascontiguousarray(w3.T * ln2_w[:, None])
    w2Tn = np.ascontiguousarray(w2.T)
    qnc = np.ascontiguousarray(q_norm_w.reshape(16, 128).T)
    knc = np.ascontiguousarray(k_norm_w.reshape(4, 128).T)
    band = _band_mask()
    ones_r = np.ones((1, 128), np.float32)
    ones_b = np.ones((128, 1), ml_dtypes.bfloat16)

    in_maps = []
    for c in range(NCORES):
        t0 = c * TOK
        bidx = t0 // S
        s0 = t0 % S
        xe = np.zeros((EXT, HID), np.float32)
        lo = s0 - WIN
        if lo >= 0:
            xe[:] = xf[bidx * S + lo: bidx * S + s0 + TOK]
            halo_valid = True
        else:
            xe[WIN:] = xf[bidx * S + s0: bidx * S + s0 + TOK]
            halo_valid = False
        xTc = np.ascontiguousarray(xe.T)

        qpos = np.arange(s0, s0 + TOK)
        kpos = np.arange(s0 - WIN, s0 + TOK)
        cq, sq_ = _rope_tables(qpos)
        ck, sk_ = _rope_tables(np.maximum(kpos, 0))
        halo_bias = np.zeros(EXT, np.float32)
        if not halo_valid:
            halo_bias[0:WIN] = NEG
        haloc = np.ascontiguousarray(halo_bias.reshape(8, 128).T)

        in_maps.append({
            "xT": xTc,
            "wqT": wqTn, "wkT": wkTn, "wvT": wvTn, "woT": woTn,
            "w1T": np.ascontiguousarray(w1Tn[:, c * IMC:(c + 1) * IMC]),
            "w3T": np.ascontiguousarray(w3Tn[:, c * IMC:(c + 1) * IMC]),
            "w2T": np.ascontiguousarray(w2Tn[c * IMC:(c + 1) * IMC, :]),
            "qnw": qnc, "knw": knc,
            "cos_q": cq, "sinS_q": sq_, "cos_k": ck, "sinS_k": sk_,
            "halo": haloc, "band": band,
            "ones_r": ones_r, "ones_b": ones_b,
        })
    return in_maps


_NC = None


def _get_nc():
    global _NC
    if _NC is None:
        _register_ntff_hook()
        _NC = build_nc()
    return _NC


def run(in_maps, trace=False):
    from concourse.bass_utils import run_bass_kernel_spmd
    nc = _get_nc()
    return run_bass_kernel_spmd(nc, in_maps, core_ids=list(range(NCORES)),
                                trace=trace)


def kernel(**inputs):
    arrs = {k: np.asarray(v, dtype=np.float32) for k, v in inputs.items()}
    in_maps = _prepare_in_maps(
        arrs["hidden_states"], arrs["wq"], arrs["wk"], arrs["wv"], arrs["wo"],
        arrs["q_norm_w"], arrs["k_norm_w"], arrs["ln1_w"], arrs["ln2_w"],
        arrs["w1"], arrs["w2"], arrs["w3"])
    res = run(in_maps, trace=False)
    full = np.empty((B * S, HID), np.float32)
    for c in range(NCORES):
        full[c * TOK:(c + 1) * TOK] = res.results[c]["out"]
    return full.reshape(B, S, HID)


# revision 13
# speedup vs baseline: 1.0087x; 1.0087x over previous
"""MiniMax-M2 decoder layer on 8 trn2 NeuronCores.

Sharding: sequence-sharded attention (each core owns 512 tokens of the
flattened (B*S)=4096 token stream and recomputes the 512-token KV halo
locally -> no collectives in the attention block), tensor-parallel MLP
(IM=8192 sharded 1024/core; AllGather of the ln2-normed activations,
ReduceScatter of the w2 partial sums in bf16).

v2 restructure vs baseline:
- no DRAM bf16 pre-cast phase: weights stream via gpsimd casting DMAs
  (f32 DRAM -> bf16 SBUF in flight), in big [128,512]+ tiles.
- w1/w3 SBUF-resident bf16, loaded once during the attention phase.
- ln1_w/ln2_w folded into wq/wk/wv/w1/w3 host-side.
- AllGather split 4-ways along own-token columns, outputs in Shared
  DRAM; MLP chunks start as soon as the first slice lands.
- MLP keeps all gate activations (gs) in SBUF; w2 runs hs-slice-major
  with a pipelined 4-way ReduceScatter + per-slice residual tail.
- x2 residual stream in bf16 (rel-err budget allows).

Self-contained: includes the BIR wait-splitting fix this container's
walrus build needs (1 semaphore wait per instruction max).
"""

import json
import sys
import types

import numpy as np

import concourse.bass as bass
import concourse.mybir as mybir
import concourse.tile as tile
from concourse.masks import make_identity

# ---------------------------------------------------------------- constants
B, S, HID = 2, 2048, 2048
H, HK, D = 16, 4, 128
RD = 64
IM = 8192
WIN = 512
EPS = 1e-6
THETA = 10000.0
SCALE = D ** -0.5

NCORES = 8
TOK = 512              # own tokens per core
EXT = 1024             # halo + own
IMC = IM // NCORES     # 1024 im rows per core
NEG = -1e9

F32 = mybir.dt.float32
F32R = mybir.dt.float32r
BF16 = mybir.dt.bfloat16
AF = mybir.ActivationFunctionType

KT = 8                 # 128-wide key tiles over EXT
NM = HID // 128        # 16 hid tiles
NMI = IMC // 128       # 8 im tiles per core
NQ = 4                 # AllGather token-column split
QW = TOK // NQ         # 128 columns per AG slice

# ------------------------------------------------------- walrus wait-split fix
MAX_WAITS = 1


def _split_excess_waits(bir_bytes: bytes) -> bytes:
    m = json.loads(bir_bytes)
    ctr = [0]

    def fix_insts(insts):
        out = []
        for ins in insts:
            si = ins.get("sync_info")
            ow = (si or {}).get("on_wait") or []
            if len(ow) > MAX_WAITS:
                eng = ins["engine"]
                keep = ow[-MAX_WAITS:]
                excess = ow[:-MAX_WAITS]
                ins["sync_info"]["on_wait"] = keep
                for i in range(0, len(excess), MAX_WAITS):
                    ctr[0] += 1
                    out.append({
                        "debug": ins.get("debug", 0),
                        "engine": eng,
                        "ins": [],
                        "name": f"I-waitfix-{ctr[0]}",
                        "opcode": "NoOp",
                        "outs": [],
                        "sync_info": {"on_update": [],
                                      "on_wait": excess[i:i + MAX_WAITS]},
                        "text_hint": "waitfix",
                    })
            out.append(ins)
        return out

    def walk(o):
        if isinstance(o, dict):
            if isinstance(o.get("instructions"), list):
                o["instructions"] = fix_insts(o["instructions"])
            for v in o.values():
                walk(v)
        elif isinstance(o, list):
            for v in o:
                walk(v)

    walk(m)
    return json.dumps(m).encode()


class _BassFixed(bass.Bass):
    def to_json_bytes(self) -> bytes:
        return _split_excess_waits(super().to_json_bytes())


def _register_ntff_hook():
    """Provide antenv.axon_hooks (missing in this image) so trace=True works."""
    if "antenv.axon_hooks" in sys.modules:
        return
    try:
        import trn_agent_boot.trn_boot as tb
    except ImportError:
        return
    mod = types.ModuleType("antenv.axon_hooks")
    holder = [None]
    mod.set_axon_ntff_profile_hook = lambda h: holder.__setitem__(0, h)
    mod.get_axon_ntff_profile_hook = lambda: holder[0]
    sys.modules["antenv.axon_hooks"] = mod
    try:
        mod.set_axon_ntff_profile_hook(
            tb._ntff_profile_via_ctypes("/opt/axon/libaxon_pjrt.so"))
    except Exception:
        pass


# ---------------------------------------------------------------- the program
def build_nc():
    nc = _BassFixed(num_devices=NCORES, target_bir_lowering=False)

    xT = nc.dram_tensor("xT", [HID, EXT], F32R, kind="ExternalInput")
    wqT = nc.dram_tensor("wqT", [HID, H * D], F32R, kind="ExternalInput")
    wkT = nc.dram_tensor("wkT", [HID, HK * D], F32R, kind="ExternalInput")
    wvT = nc.dram_tensor("wvT", [HID, HK * D], F32R, kind="ExternalInput")
    woT = nc.dram_tensor("woT", [H * D, HID], F32R, kind="ExternalInput")
    w1T = nc.dram_tensor("w1T", [HID, IMC], F32R, kind="ExternalInput")
    w3T = nc.dram_tensor("w3T", [HID, IMC], F32R, kind="ExternalInput")
    w2T = nc.dram_tensor("w2T", [IMC, HID], F32R, kind="ExternalInput")
    qnw = nc.dram_tensor("qnw", [128, 16], F32, kind="ExternalInput")
    knw = nc.dram_tensor("knw", [128, 4], F32, kind="ExternalInput")
    cos_q = nc.dram_tensor("cos_q", [RD, TOK], F32, kind="ExternalInput")
    sinS_q = nc.dram_tensor("sinS_q", [RD, TOK], F32, kind="ExternalInput")
    cos_k = nc.dram_tensor("cos_k", [RD, EXT], F32, kind="ExternalInput")
    sinS_k = nc.dram_tensor("sinS_k", [RD, EXT], F32, kind="ExternalInput")
    halo = nc.dram_tensor("halo", [128, 8], F32, kind="ExternalInput")
    band = nc.dram_tensor("band", [128, 1408], BF16, kind="ExternalInput")
    ones_r = nc.dram_tensor("ones_r", [1, 128], F32R, kind="ExternalInput")
    ones_b = nc.dram_tensor("ones_b", [128, 1], BF16, kind="ExternalInput")

    out = nc.dram_tensor("out", [TOK, HID], F32, kind="ExternalOutput")

    with tile.TileContext(nc) as tc:
        with tc.tile_pool(name="consts", bufs=1) as cst, \
             tc.tile_pool(name="res", bufs=1) as res, \
             tc.tile_pool(name="dram", bufs=1, space="DRAM") as dram, \
             tc.tile_pool(name="rowps", bufs=3, space="PSUM") as rps, \
             tc.tile_pool(name="bcps", bufs=1, space="PSUM") as bcps:

            # ---------------- constants
            oner = cst.tile([1, 128], F32R)
            nc.sync.dma_start(oner[:], ones_r[:])
            oneb = cst.tile([128, 1], BF16)
            nc.sync.dma_start(oneb[:], ones_b[:])
            qnw_s = cst.tile([128, 16], F32)
            nc.sync.dma_start(qnw_s[:], qnw[:])
            knw_s = cst.tile([128, 4], F32)
            nc.sync.dma_start(knw_s[:], knw[:])
            halo_s = cst.tile([128, 8], F32)
            nc.sync.dma_start(halo_s[:], halo[:])
            band_s = cst.tile([128, 1408], BF16)
            nc.sync.dma_start(band_s[:], band[:])
            eps_s = cst.tile([1, 1], F32)
            nc.vector.memset(eps_s[:], EPS)
            identF = cst.tile([128, 128], F32)
            make_identity(nc, identF[:])
            identB = cst.tile([128, 128], BF16)
            nc.vector.tensor_copy(identB[:], identF[:])

            # tiny warm-up collective: absorbs the one-time CC entry
            # barrier (~40us) under the projection phase
            dummy_i = dram.tile([128, 16], BF16, name="dummy_i")
            dummy_o = dram.tile([NCORES, 128, 16], BF16,
                                addr_space="Shared", name="dummy_o")
            dmv = cst.tile([128, 16], BF16)
            nc.vector.memset(dmv[:], 0.0)
            nc.sync.dma_start(dummy_i[:], dmv[:])
            nc.gpsimd.collective_compute(
                "AllGather", mybir.AluOpType.bypass,
                replica_groups=[list(range(NCORES))],
                ins=[dummy_i.opt()], outs=[dummy_o.opt()],
            )

            # resident MLP weights (loads issued after QKV weight streaming)
            w1s = res.tile([128, NM, IMC], BF16)
            w3s = res.tile([128, NM, IMC], BF16)

            # internal DRAM
            ag_in = [dram.tile([HID, QW], BF16, name=f"ag_in{q}")
                     for q in range(NQ)]
            ag_out = [dram.tile([NCORES, HID, QW], BF16, addr_space="Shared",
                                name=f"ag_out{q}") for q in range(NQ)]
            rs_in = [dram.tile([NCORES * TOK, 512], BF16, name=f"rs_in{h}")
                     for h in range(4)]
            rs_out = [dram.tile([TOK, 512], BF16, name=f"rs_out{h}")
                      for h in range(4)]

            # ======== attention block scope
            with tc.tile_pool(name="qkv", bufs=1) as qkv, \
                 tc.tile_pool(name="smalls", bufs=1) as sml:
                qT = qkv.tile([128, H, TOK], BF16)
                kT = qkv.tile([128, HK, EXT], BF16)
                Vb = qkv.tile([128, KT, HK * D], BF16)

                def bcast_row(row_f32r, width, tag):
                    """[1,width] F32R row -> list of (psum[128,w], j, w)."""
                    parts = []
                    for j in range(0, width, 512):
                        w = min(512, width - j)
                        p = bcps.tile([128, w], F32, tag="bc",
                                      name=f"bc_{tag}_{j}")
                        nc.tensor.matmul(p[:], oner[:],
                                         row_f32r[:, j:j + w],
                                         start=True, stop=True)
                        parts.append((p, j, w))
                    return parts

                def row_rsqrt_bcast(acc_parts, width, denom, tag):
                    """psum [1,*] sum-of-squares parts -> [128,*] psum
                    chunks with 1/sqrt(acc/denom + eps) bcast."""
                    srow = sml.tile([1, width], F32, tag="srow")
                    for (p, j, w) in acc_parts:
                        nc.scalar.activation(out=srow[:, j:j + w], in_=p[:],
                                             func=AF.Sqrt, bias=eps_s[:],
                                             scale=1.0 / denom)
                    rrow = sml.tile([1, width], F32R, tag="rrow")
                    with nc.allow_low_precision(reason="f32r intended"):
                        nc.vector.reciprocal(rrow[:], srow[:])
                    return bcast_row(rrow, width, tag)

                # ============ projections, two 512-token halves (own first)
                with tc.tile_pool(name="hp", bufs=1) as hp, \
                     tc.tile_pool(name="nrm", bufs=1) as nrm, \
                     tc.tile_pool(name="xs", bufs=3) as xs, \
                     tc.tile_pool(name="sqp", bufs=3) as sqp, \
                     tc.tile_pool(name="ws", bufs=3) as ws, \
                     tc.tile_pool(name="accP", bufs=4, space="PSUM") as accP:
                    # rope tables
                    cq_s = nrm.tile([RD, TOK], F32)
                    nc.sync.dma_start(cq_s[:], cos_q[:])
                    sq_s = nrm.tile([RD, TOK], F32)
                    nc.sync.dma_start(sq_s[:], sinS_q[:])
                    ck_s = nrm.tile([RD, EXT], F32)
                    nc.sync.dma_start(ck_s[:], cos_k[:])
                    sk_s = nrm.tile([RD, EXT], F32)
                    nc.sync.dma_start(sk_s[:], sinS_k[:])

                    def rope(t3, nh, width, cos_t, sinS_t):
                        c3 = cos_t[:].rearrange(
                            "p (g t) -> p g t",
                            g=1).broadcast_to([RD, nh, width])
                        s3 = sinS_t[:].rearrange(
                            "p (g t) -> p g t",
                            g=1).broadcast_to([RD, nh, width])
                        # partner halves moved onto matching partitions
                        qsw = nrm.tile([RD, nh, width], BF16, tag="rsw")
                        nc.sync.dma_start(qsw[0:32], t3[32:64])
                        nc.sync.dma_start(qsw[32:64], t3[0:32])
                        t1 = nrm.tile([RD, nh, width], BF16, tag="rt1")
                        nc.vector.tensor_mul(t1[:], t3[0:RD], c3)
                        nc.vector.tensor_mul(qsw[:], qsw[:], s3)
                        nc.vector.tensor_add(t3[0:RD], t1[:], qsw[:])

                    vT = hp.tile([128, HK, EXT], BF16, tag="vT")
                    for half in (1, 0):
                        c0 = half * 512
                        acc = rps.tile([1, 512], F32, tag="row")
                        for i in range(NM):
                            xt = xs.tile([128, 512], F32R, tag="xt")
                            nc.sync.dma_start(
                                xt[:], xT[i * 128:(i + 1) * 128, c0:c0 + 512])
                            sq = sqp.tile([128, 512], BF16, tag="sq")
                            nc.vector.tensor_mul(sq[:], xt[:], xt[:])
                            nc.tensor.matmul(acc[:], oneb[:], sq[:],
                                             start=(i == 0), stop=(i == NM - 1))
                        s1b = row_rsqrt_bcast([(acc, 0, 512)], 512, HID,
                                               "l1")[0][0]
                        hTh = hp.tile([128, NM, 512], BF16, tag="h")
                        for i in range(NM):
                            xt = xs.tile([128, 512], F32R, tag="xt")
                            nc.sync.dma_start(
                                xt[:], xT[i * 128:(i + 1) * 128, c0:c0 + 512])
                            nc.vector.tensor_mul(hTh[:, i, :], xt[:], s1b[:])

                        if half == 1:
                            # Q projection (own tokens only), 4-head groups
                            for mg in range(4):
                                pq = [accP.tile([128, 512], F32, tag="acc",
                                                 name=f"pq{j}")
                                      for j in range(4)]
                                for i in range(NM):
                                    wq_t = ws.tile([128, 512], BF16, tag="wq")
                                    nc.gpsimd.dma_start(
                                        wq_t[:],
                                        wqT[i * 128:(i + 1) * 128,
                                            mg * 512:(mg + 1) * 512])
                                    for j in range(4):
                                        nc.tensor.matmul(
                                            pq[j][:],
                                            wq_t[:, j * 128:(j + 1) * 128],
                                            hTh[:, i, :],
                                            start=(i == 0), stop=(i == NM - 1))
                                for j in range(4):
                                    nc.vector.tensor_copy(
                                        qT[:, mg * 4 + j, :], pq[j][:])

                        # K / V for this half (i-outer, 4 kv heads inner)
                        pk = [accP.tile([128, 512], F32, tag="acc",
                                         name=f"pk{g}")
                              for g in range(4)]
                        for i in range(NM):
                            wk_t = ws.tile([128, 512], BF16, tag="wk")
                            nc.gpsimd.dma_start(
                                wk_t[:], wkT[i * 128:(i + 1) * 128, :])
                            for g in range(HK):
                                nc.tensor.matmul(
                                    pk[g][:], wk_t[:, g * 128:(g + 1) * 128],
                                    hTh[:, i, :],
                                    start=(i == 0), stop=(i == NM - 1))
                        for g in range(HK):
                            nc.vector.tensor_copy(kT[:, g, c0:c0 + 512],
                                                  pk[g][:])
                        pv = [accP.tile([128, 512], F32, tag="acc",
                                         name=f"pv{g}")
                              for g in range(4)]
                        for i in range(NM):
                            wv_t = ws.tile([128, 512], BF16, tag="wv")
                            nc.gpsimd.dma_start(
                                wv_t[:], wvT[i * 128:(i + 1) * 128, :])
                            for g in range(HK):
                                nc.tensor.matmul(
                                    pv[g][:], wv_t[:, g * 128:(g + 1) * 128],
                                    hTh[:, i, :],
                                    start=(i == 0), stop=(i == NM - 1))
                        for g in range(HK):
                            nc.vector.tensor_copy(vT[:, g, c0:c0 + 512],
                                                  pv[g][:])

                        if half == 1:
                            # fused q RMSNorm + rope; overlaps half-0 K/V
                            accq = rps.tile([1, 512], F32, tag="row")
                            sqq = nrm.tile([128, TOK], BF16, tag="nsq")
                            for h in range(H):
                                nc.vector.tensor_mul(sqq[:], qT[:, h, :],
                                                     qT[:, h, :])
                                nc.tensor.matmul(accq[:], oneb[:], sqq[:],
                                                 start=(h == 0),
                                                 stop=(h == H - 1))
                            cqb = row_rsqrt_bcast([(accq, 0, 512)], TOK,
                                                  H * D, "qn")[0][0]
                            cqb3 = cqb[:].rearrange(
                                "p (g t) -> p g t",
                                g=1).broadcast_to([128, H, TOK])
                            nc.vector.tensor_mul(qT[:], qT[:], cqb3)
                            for h in range(H):
                                nc.vector.tensor_scalar_mul(
                                    qT[:, h, :], qT[:, h, :],
                                    qnw_s[:, h:h + 1])
                            rope(qT[:, 0:8, :], 8, TOK, cq_s, sq_s)
                            rope(qT[:, 8:16, :], 8, TOK, cq_s, sq_s)

                    # resident MLP weight loads (gpsimd queue, after qkv w)
                    for i in range(NM):
                        nc.gpsimd.dma_start(
                            w1s[:, i, :], w1T[i * 128:(i + 1) * 128, :])
                    for i in range(NM):
                        nc.gpsimd.dma_start(
                            w3s[:, i, :], w3T[i * 128:(i + 1) * 128, :])

                    # k RMSNorm + rope
                    acck_lo = rps.tile([1, 512], F32, tag="row")
                    acck_hi = rps.tile([1, 512], F32, tag="row")
                    sqk = nrm.tile([128, EXT], BF16, tag="nsqk")
                    for g in range(HK):
                        nc.vector.tensor_mul(sqk[:], kT[:, g, :], kT[:, g, :])
                        nc.tensor.matmul(acck_lo[:], oneb[:], sqk[:, 0:512],
                                         start=(g == 0), stop=(g == HK - 1))
                        nc.tensor.matmul(acck_hi[:], oneb[:], sqk[:, 512:1024],
                                         start=(g == 0), stop=(g == HK - 1))
                    ckb_parts = row_rsqrt_bcast(
                        [(acck_lo, 0, 512), (acck_hi, 512, 512)],
                        EXT, HK * D, "kn")
                    for (pck, j, w) in ckb_parts:
                        v = pck[:].rearrange(
                            "p (g t) -> p g t",
                            g=1).broadcast_to([128, HK, w])
                        nc.vector.tensor_mul(kT[:, :, j:j + w],
                                             kT[:, :, j:j + w], v)
                    for g in range(HK):
                        nc.vector.tensor_scalar_mul(
                            kT[:, g, :], kT[:, g, :], knw_s[:, g:g + 1])
                    rope(kT[:], HK, EXT, ck_s, sk_s)

                    # transpose vT -> token-major bf16 Vb
                    for kt in range(KT):
                        for g in range(HK):
                            pt = accP.tile([128, 128], BF16, tag="acc")
                            nc.tensor.transpose(
                                pt[:], vT[:, g, kt * 128:(kt + 1) * 128],
                                identB[:])
                            nc.vector.tensor_copy(
                                Vb[:, kt, g * 128:(g + 1) * 128], pt[:])

                # ============ sliding-window attention
                with tc.tile_pool(name="attn", bufs=1) as ap, \
                     tc.tile_pool(name="es", bufs=12) as es, \
                     tc.tile_pool(name="bps", bufs=4, space="PSUM") as bps:
                    attnT = ap.tile([128, H, TOK], BF16)
                    for h in range(H):
                        g = h // (H // HK)
                        exps = []
                        for kt in range(KT):
                            ps = bps.tile([128, 512], F32, tag="big")
                            nc.tensor.matmul(
                                ps[:], kT[:, g, kt * 128:(kt + 1) * 128],
                                qT[:, h, :], start=True, stop=True)
                            e = es.tile([128, 512], BF16, tag="e")
                            nc.scalar.activation(
                                out=e[:], in_=ps[:], func=AF.Exp,
                                bias=halo_s[:, kt:kt + 1], scale=SCALE)
                            nc.vector.tensor_mul(
                                e[:], e[:],
                                band_s[:, 896 - 128 * kt:1408 - 128 * kt])
                            exps.append(e)
                        den = rps.tile([1, 512], F32, tag="row")
                        for kt in range(KT):
                            nc.tensor.matmul(den[:], oneb[:], exps[kt][:],
                                             start=(kt == 0),
                                             stop=(kt == KT - 1))
                        dr = sml.tile([1, 512], F32, tag="dr")
                        nc.vector.tensor_copy(dr[:], den[:])
                        drr = sml.tile([1, 512], F32R, tag="drr")
                        with nc.allow_low_precision(reason="f32r intended"):
                            nc.vector.reciprocal(drr[:], dr[:])
                        rb = bcast_row(drr, 512, "rden")[0][0]
                        rbs = sml.tile([128, 512], F32R, tag="rbs")
                        nc.vector.tensor_copy(rbs[:], rb[:])
                        po = bps.tile([128, 512], F32, tag="big")
                        for kt in range(KT):
                            nc.tensor.matmul(
                                po[:],
                                Vb[:, kt, g * 128:(g + 1) * 128],
                                exps[kt][:], start=(kt == 0),
                                stop=(kt == KT - 1))
                        nc.vector.tensor_mul(attnT[:, h, :], po[:], rbs[:])

                    # ============ o_proj + residual + ln2
                    with tc.tile_pool(name="x2", bufs=1) as x2p, \
                         tc.tile_pool(name="xs2", bufs=3) as xs2:
                        x2T = x2p.tile([128, NM, TOK], BF16)
                        acc2 = rps.tile([1, 512], F32, tag="row")
                        for mg in range(4):
                            px = [bps.tile([128, 512], F32, tag="big",
                                            name=f"px{j}")
                                  for j in range(4)]
                            for i in range(NM):
                                wo_t = xs2.tile([128, 512], BF16, tag="wo")
                                nc.gpsimd.dma_start(
                                    wo_t[:], woT[i * 128:(i + 1) * 128,
                                                 mg * 512:(mg + 1) * 512])
                                for j in range(4):
                                    nc.tensor.matmul(
                                        px[j][:],
                                        wo_t[:, j * 128:(j + 1) * 128],
                                        attnT[:, i, :],
                                        start=(i == 0), stop=(i == NM - 1))
                            for j in range(4):
                                m = mg * 4 + j
                                xo = xs2.tile([128, TOK], F32R, tag="xo")
                                nc.sync.dma_start(
                                    xo[:], xT[m * 128:(m + 1) * 128, 512:1024])
                                nc.vector.tensor_add(x2T[:, m, :], px[j][:],
                                                     xo[:])
                                sq2 = xs2.tile([128, TOK], BF16, tag="sq2")
                                nc.vector.tensor_mul(sq2[:], x2T[:, m, :],
                                                     x2T[:, m, :])
                                nc.tensor.matmul(acc2[:], oneb[:], sq2[:],
                                                 start=(m == 0),
                                                 stop=(m == NM - 1))

                        # ln2 -> h2 -> AG inputs (first, so AG starts early)
                        s2b = row_rsqrt_bcast([(acc2, 0, 512)], TOK, HID,
                                                "l2")[0][0]
                        for m in range(NM):
                            h2t = xs2.tile([128, TOK], BF16, tag="h2t")
                            nc.vector.tensor_mul(h2t[:], x2T[:, m, :], s2b[:])
                            for q in range(NQ):
                                eng = nc.sync if q < 2 else nc.scalar
                                eng.dma_start(
                                    ag_in[q][m * 128:(m + 1) * 128, :],
                                    h2t[:, q * QW:(q + 1) * QW])

                        for q in range(NQ):
                            nc.gpsimd.collective_compute(
                                "AllGather", mybir.AluOpType.bypass,
                                replica_groups=[list(range(NCORES))],
                                ins=[ag_in[q].opt()], outs=[ag_out[q].opt()],
                            )

                        # x2 token-major -> DRAM (overlaps the AllGather)
                        for tt in range(4):
                            for grp in range(4):
                                ts = xs2.tile([128, 512], BF16, tag="x2t")
                                for j in range(4):
                                    m = grp * 4 + j
                                    pt = bps.tile([128, 128], BF16, tag="big")
                                    nc.tensor.transpose(
                                        pt[:],
                                        x2T[:, m, tt * 128:(tt + 1) * 128],
                                        identB[:])
                                    nc.vector.tensor_copy(
                                        ts[:, j * 128:(j + 1) * 128], pt[:])
                                nc.gpsimd.dma_start(
                                    out[tt * 128:(tt + 1) * 128,
                                        grp * 512:(grp + 1) * 512], ts[:])

            # ============ TP MLP: chunks of 512 tokens (4 cores x 128 cols)
            with tc.tile_pool(name="gsp", bufs=1) as gsp, \
                 tc.tile_pool(name="mh", bufs=2) as mh, \
                 tc.tile_pool(name="silp", bufs=2) as silp, \
                 tc.tile_pool(name="w2s", bufs=2) as w2sp, \
                 tc.tile_pool(name="po", bufs=2) as pop, \
                 tc.tile_pool(name="mps", bufs=4, space="PSUM") as mps:
                gs = gsp.tile([128, NMI, NCORES * TOK], BF16)
                for q in range(NQ):
                    for quad in range(2):
                        h2c = mh.tile([128, NM, 512], BF16, tag="h2")
                        for i in range(NM):
                            src = ag_out[q][quad * 4:(quad + 1) * 4,
                                            i * 128:(i + 1) * 128, :]
                            nc.sync.dma_start(
                                h2c[:, i, :].rearrange(
                                    "p (c w) -> p c w", c=4),
                                src.rearrange("c p w -> p c w"))
                        for m in range(NMI):
                            a = mps.tile([128, 512], F32, tag="m")
                            for i in range(NM):
                                nc.tensor.matmul(
                                    a[:], w1s[:, i, m * 128:(m + 1) * 128],
                                    h2c[:, i, :],
                                    start=(i == 0), stop=(i == NM - 1))
                            sil = silp.tile([128, 512], BF16, tag="sil")
                            nc.scalar.activation(out=sil[:], in_=a[:],
                                                 func=AF.Silu)
                            b = mps.tile([128, 512], F32, tag="m")
                            for i in range(NM):
                                nc.tensor.matmul(
                                    b[:], w3s[:, i, m * 128:(m + 1) * 128],
                                    h2c[:, i, :],
                                    start=(i == 0), stop=(i == NM - 1))
                            dst = gs[:, m,
                                     quad * 4 * TOK:(quad * 4 + 4) * TOK]
                            dst3 = dst.rearrange("p (c w) -> p c w", c=4)
                            nc.vector.tensor_mul(
                                dst3[:, :, q * QW:(q + 1) * QW],
                                sil[:].rearrange("p (c w) -> p c w", c=4),
                                b[:].rearrange("p (c w) -> p c w", c=4))

                # w2 by output-column slice; pipelined ReduceScatter + tail
                w2tiles = {}

                def load_w2(hs):
                    t = w2sp.tile([128, NMI, 512], BF16, tag="w2c",
                                  name=f"w2c{hs}")
                    for m in range(NMI):
                        nc.gpsimd.dma_start(
                            t[:, m, :], w2T[m * 128:(m + 1) * 128,
                                            hs * 512:(hs + 1) * 512])
                    w2tiles[hs] = t

                load_w2(0)
                load_w2(1)
                for hs in range(4):
                    w2c = w2tiles[hs]
                    for tt in range(NCORES * TOK // 128):
                        op_ = mps.tile([128, 512], F32, tag="m")
                        for m in range(NMI):
                            nc.tensor.matmul(
                                op_[:], gs[:, m, tt * 128:(tt + 1) * 128],
                                w2c[:, m, :],
                                start=(m == 0), stop=(m == NMI - 1))
                        pb = pop.tile([128, 512], BF16, tag="pb")
                        nc.vector.tensor_copy(pb[:], op_[:])
                        nc.sync.dma_start(
                            rs_in[hs][tt * 128:(tt + 1) * 128, :], pb[:])
                    if hs + 2 < 4:
                        load_w2(hs + 2)
                    nc.gpsimd.collective_compute(
                        "ReduceScatter", mybir.AluOpType.add,
                        replica_groups=[list(range(NCORES))],
                        ins=[rs_in[hs].opt()], outs=[rs_out[hs].opt()],
                    )

                with tc.tile_pool(name="tail", bufs=4) as tp:
                    for hs in range(4):
                        for tt in range(4):
                            rsf = tp.tile([128, 512], F32, tag="rsf")
                            nc.gpsimd.dma_start(
                                rsf[:], rs_out[hs][tt * 128:(tt + 1) * 128, :])
                            nc.gpsimd.dma_start(
                                out[tt * 128:(tt + 1) * 128,
                                    hs * 512:(hs + 1) * 512], rsf[:],
                                accum_op=mybir.AluOpType.add)

    return nc


# ---------------------------------------------------------------- host side
def _rope_tables(pos):
    inv = 1.0 / (THETA ** (np.arange(0, RD, 2, dtype=np.float32) / RD))
    f = pos[:, None].astype(np.float32) * inv[None, :]
    emb = np.concatenate([f, f], axis=-1)          # [T, RD]
    cos = np.ascontiguousarray(np.cos(emb).T)      # [RD, T]
    sin = np.sin(emb).T
    sinS = sin.copy()
    sinS[0:32] = -sin[0:32]
    return cos.astype(np.float32), np.ascontiguousarray(sinS).astype(np.float32)


def _band_mask():
    import ml_dtypes
    p = np.arange(128)[:, None]
    u = np.arange(1408)[None, :]
    m = ((u >= p + 384) & (u <= p + 896)).astype(np.float32)
    return m.astype(ml_dtypes.bfloat16)


def _prepare_in_maps(hidden_states, wq, wk, wv, wo, q_norm_w, k_norm_w,
                     ln1_w, ln2_w, w1, w2, w3):
    import ml_dtypes
    xf = np.ascontiguousarray(hidden_states.reshape(B * S, HID))
    # fold ln1_w into wq/wk/wv rows, ln2_w into w1/w3 rows
    wqTn = np.ascontiguousarray(wq.T * ln1_w[:, None])
    wkTn = np.ascontiguousarray(wk.T * ln1_w[:, None])
    wvTn = np.ascontiguousarray(wv.T * ln1_w[:, None])
    woTn = np.ascontiguousarray(wo.T)
    w1Tn = np.ascontiguousarray(w1.T * ln2_w[:, None])
    w3Tn = np.ascontiguousarray(w3.T * ln2_w[:, None])
    w2Tn = np.ascontiguousarray(w2.T)
    qnc = np.ascontiguousarray(q_norm_w.reshape(16, 128).T)
    knc = np.ascontiguousarray(k_norm_w.reshape(4, 128).T)
    band = _band_mask()
    ones_r = np.ones((1, 128), np.float32)
    ones_b = np.ones((128, 1), ml_dtypes.bfloat16)

    in_maps = []
    for c in range(NCORES):
        t0 = c * TOK
        bidx = t0 // S
        s0 = t0 % S
        xe = np.zeros((EXT, HID), np.float32)
        lo = s0 - WIN
        if lo >= 0:
            xe[:] = xf[bidx * S + lo: bidx * S + s0 + TOK]
            halo_valid = True
        else:
            xe[WIN:] = xf[bidx * S + s0: bidx * S + s0 + TOK]
            halo_valid = False
        xTc = np.ascontiguousarray(xe.T)

        qpos = np.arange(s0, s0 + TOK)
        kpos = np.arange(s0 - WIN, s0 + TOK)
        cq, sq_ = _rope_tables(qpos)
        ck, sk_ = _rope_tables(np.maximum(kpos, 0))
        halo_bias = np.zeros(EXT, np.float32)
        if not halo_valid:
            halo_bias[0:WIN] = NEG
        haloc = np.ascontiguousarray(halo_bias.reshape(8, 128).T)

        in_maps.append({
            "xT": xTc,
            "wqT": wqTn, "wkT": wkTn, "wvT": wvTn, "woT": woTn,
            "w1T": np.ascontiguousarray(w1Tn[:, c * IMC:(c + 1) * IMC]),
            "w3T": np.ascontiguousarray(w3Tn[:, c * IMC:(c + 1) * IMC]),
            "w2T": np.ascontiguousarray(w2Tn[c * IMC:(c + 1) * IMC, :]),
            "qnw": qnc, "knw": knc,
            "cos_q": cq, "sinS_q": sq_, "cos_k": ck, "sinS_k": sk_,
            "halo": haloc, "band": band,
            "ones_r": ones_r, "ones_b": ones_b,
        })
    return in_maps


_NC = None


def _get_nc():
    global _NC
    if _NC is None:
        _register_ntff_hook()
        _NC = build_nc()
    return _NC


def run(in_maps, trace=False):
    from concourse.bass_utils import run_bass_kernel_spmd
    nc = _get_nc()
    return run_bass_kernel_spmd(nc, in_maps, core_ids=list(range(NCORES)),
                                trace=trace)


def kernel(**inputs):
    arrs = {k: np.asarray(v, dtype=np.float32) for k, v in inputs.items()}
    in_maps = _prepare_in_maps(
        arrs["hidden_states"], arrs["wq"], arrs["wk"], arrs["wv"], arrs["wo"],
        arrs["q_norm_w"], arrs["k_norm_w"], arrs["ln1_w"], arrs["ln2_w"],
        arrs["w1"], arrs["w2"], arrs["w3"])
    res = run(in_maps, trace=False)
    full = np.empty((B * S, HID), np.float32)
    for c in range(NCORES):
        full[c * TOK:(c + 1) * TOK] = res.results[c]["out"]
    return full.reshape(B, S, HID)


# revision 14
# speedup vs baseline: 1.1083x; 1.0988x over previous
"""MiniMax-M2 decoder layer on 8 trn2 NeuronCores.

Sharding: sequence-sharded attention (each core owns 512 tokens of the
flattened (B*S)=4096 token stream and recomputes the 512-token KV halo
locally -> no collectives in the attention block), tensor-parallel MLP
(IM=8192 sharded 1024/core; AllGather of the ln2-normed activations,
ReduceScatter of the w2 partial sums in bf16).

v2 restructure vs baseline:
- no DRAM bf16 pre-cast phase: weights stream via gpsimd casting DMAs
  (f32 DRAM -> bf16 SBUF in flight), in big [128,512]+ tiles.
- w1/w3 SBUF-resident bf16, loaded once during the attention phase.
- ln1_w/ln2_w folded into wq/wk/wv/w1/w3 host-side.
- AllGather split 4-ways along own-token columns, outputs in Shared
  DRAM; MLP chunks start as soon as the first slice lands.
- MLP keeps all gate activations (gs) in SBUF; w2 runs hs-slice-major
  with a pipelined 4-way ReduceScatter + per-slice residual tail.
- x2 residual stream in bf16 (rel-err budget allows).

Self-contained: includes the BIR wait-splitting fix this container's
walrus build needs (1 semaphore wait per instruction max).
"""

import json
import sys
import types

import numpy as np

import concourse.bass as bass
import concourse.mybir as mybir
import concourse.tile as tile
from concourse.masks import make_identity

# ---------------------------------------------------------------- constants
B, S, HID = 2, 2048, 2048
H, HK, D = 16, 4, 128
RD = 64
IM = 8192
WIN = 512
EPS = 1e-6
THETA = 10000.0
SCALE = D ** -0.5

NCORES = 8
TOK = 512              # own tokens per core
EXT = 1024             # halo + own
IMC = IM // NCORES     # 1024 im rows per core
NEG = -1e9

F32 = mybir.dt.float32
F32R = mybir.dt.float32r
BF16 = mybir.dt.bfloat16
AF = mybir.ActivationFunctionType

KT = 8                 # 128-wide key tiles over EXT
NM = HID // 128        # 16 hid tiles
NMI = IMC // 128       # 8 im tiles per core
NQ = 4                 # AllGather token-column split
QW = TOK // NQ         # 128 columns per AG slice

# ------------------------------------------------------- walrus wait-split fix
MAX_WAITS = 1


def _split_excess_waits(bir_bytes: bytes) -> bytes:
    m = json.loads(bir_bytes)
    ctr = [0]

    def fix_insts(insts):
        out = []
        for ins in insts:
            si = ins.get("sync_info")
            ow = (si or {}).get("on_wait") or []
            if len(ow) > MAX_WAITS:
                eng = ins["engine"]
                keep = ow[-MAX_WAITS:]
                excess = ow[:-MAX_WAITS]
                ins["sync_info"]["on_wait"] = keep
                for i in range(0, len(excess), MAX_WAITS):
                    ctr[0] += 1
                    out.append({
                        "debug": ins.get("debug", 0),
                        "engine": eng,
                        "ins": [],
                        "name": f"I-waitfix-{ctr[0]}",
                        "opcode": "NoOp",
                        "outs": [],
                        "sync_info": {"on_update": [],
                                      "on_wait": excess[i:i + MAX_WAITS]},
                        "text_hint": "waitfix",
                    })
            out.append(ins)
        return out

    def walk(o):
        if isinstance(o, dict):
            if isinstance(o.get("instructions"), list):
                o["instructions"] = fix_insts(o["instructions"])
            for v in o.values():
                walk(v)
        elif isinstance(o, list):
            for v in o:
                walk(v)

    walk(m)
    return json.dumps(m).encode()


class _BassFixed(bass.Bass):
    def to_json_bytes(self) -> bytes:
        return _split_excess_waits(super().to_json_bytes())


def _register_ntff_hook():
    """Provide antenv.axon_hooks (missing in this image) so trace=True works."""
    if "antenv.axon_hooks" in sys.modules:
        return
    try:
        import trn_agent_boot.trn_boot as tb
    except ImportError:
        return
    mod = types.ModuleType("antenv.axon_hooks")
    holder = [None]
    mod.set_axon_ntff_profile_hook = lambda h: holder.__setitem__(0, h)
    mod.get_axon_ntff_profile_hook = lambda: holder[0]
    sys.modules["antenv.axon_hooks"] = mod
    try:
        mod.set_axon_ntff_profile_hook(
            tb._ntff_profile_via_ctypes("/opt/axon/libaxon_pjrt.so"))
    except Exception:
        pass


# ---------------------------------------------------------------- the program
def build_nc():
    nc = _BassFixed(num_devices=NCORES, target_bir_lowering=False)

    xT = nc.dram_tensor("xT", [HID, EXT], F32R, kind="ExternalInput")
    wqT = nc.dram_tensor("wqT", [HID, H * D], F32R, kind="ExternalInput")
    wkT = nc.dram_tensor("wkT", [HID, HK * D], F32R, kind="ExternalInput")
    wvT = nc.dram_tensor("wvT", [HID, HK * D], F32R, kind="ExternalInput")
    woT = nc.dram_tensor("woT", [H * D, HID], F32R, kind="ExternalInput")
    w1T = nc.dram_tensor("w1T", [HID, IMC], F32R, kind="ExternalInput")
    w3T = nc.dram_tensor("w3T", [HID, IMC], F32R, kind="ExternalInput")
    w2T = nc.dram_tensor("w2T", [IMC, HID], F32R, kind="ExternalInput")
    qnw = nc.dram_tensor("qnw", [128, 16], F32, kind="ExternalInput")
    knw = nc.dram_tensor("knw", [128, 4], F32, kind="ExternalInput")
    cos_q = nc.dram_tensor("cos_q", [RD, TOK], F32, kind="ExternalInput")
    sinS_q = nc.dram_tensor("sinS_q", [RD, TOK], F32, kind="ExternalInput")
    cos_k = nc.dram_tensor("cos_k", [RD, EXT], F32, kind="ExternalInput")
    sinS_k = nc.dram_tensor("sinS_k", [RD, EXT], F32, kind="ExternalInput")
    halo = nc.dram_tensor("halo", [128, 8], F32, kind="ExternalInput")
    band = nc.dram_tensor("band", [128, 1408], BF16, kind="ExternalInput")
    ones_r = nc.dram_tensor("ones_r", [1, 128], F32R, kind="ExternalInput")
    ones_b = nc.dram_tensor("ones_b", [128, 1], BF16, kind="ExternalInput")

    out = nc.dram_tensor("out", [TOK, HID], F32, kind="ExternalOutput")

    with tile.TileContext(nc) as tc:
        with tc.tile_pool(name="consts", bufs=1) as cst, \
             tc.tile_pool(name="res", bufs=1) as res, \
             tc.tile_pool(name="dram", bufs=1, space="DRAM") as dram, \
             tc.tile_pool(name="rowps", bufs=3, space="PSUM") as rps, \
             tc.tile_pool(name="bcps", bufs=1, space="PSUM") as bcps:

            # ---------------- constants
            oner = cst.tile([1, 128], F32R)
            nc.sync.dma_start(oner[:], ones_r[:])
            oneb = cst.tile([128, 1], BF16)
            nc.sync.dma_start(oneb[:], ones_b[:])
            qnw_s = cst.tile([128, 16], F32)
            nc.sync.dma_start(qnw_s[:], qnw[:])
            knw_s = cst.tile([128, 4], F32)
            nc.sync.dma_start(knw_s[:], knw[:])
            halo_s = cst.tile([128, 8], F32)
            nc.sync.dma_start(halo_s[:], halo[:])
            band_s = cst.tile([128, 1408], BF16)
            nc.sync.dma_start(band_s[:], band[:])
            eps_s = cst.tile([1, 1], F32)
            nc.vector.memset(eps_s[:], EPS)
            identF = cst.tile([128, 128], F32)
            make_identity(nc, identF[:])
            identB = cst.tile([128, 128], BF16)
            nc.vector.tensor_copy(identB[:], identF[:])

            # tiny warm-up collective: absorbs the one-time CC entry
            # barrier (~40us) under the projection phase
            dummy_i = dram.tile([128, 16], BF16, name="dummy_i")
            dummy_o = dram.tile([NCORES, 128, 16], BF16,
                                addr_space="Shared", name="dummy_o")
            dmv = cst.tile([128, 16], BF16)
            nc.vector.memset(dmv[:], 0.0)
            nc.sync.dma_start(dummy_i[:], dmv[:])
            nc.gpsimd.collective_compute(
                "AllGather", mybir.AluOpType.bypass,
                replica_groups=[list(range(NCORES))],
                ins=[dummy_i.opt()], outs=[dummy_o.opt()],
            )

            # resident MLP weights (loads issued after QKV weight streaming)
            w1s = res.tile([128, NM, IMC], BF16)
            w3s = res.tile([128, NM, IMC], BF16)

            # internal DRAM
            ag_in = [dram.tile([HID, QW], BF16, name=f"ag_in{q}")
                     for q in range(NQ)]
            ag_out = [dram.tile([NCORES, HID, QW], BF16, addr_space="Shared",
                                name=f"ag_out{q}") for q in range(NQ)]
            rs_in = [dram.tile([NCORES * TOK, 512], BF16, name=f"rs_in{h}")
                     for h in range(4)]
            rs_out = [dram.tile([TOK, 512], BF16, name=f"rs_out{h}")
                      for h in range(4)]

            # ======== attention block scope
            with tc.tile_pool(name="qkv", bufs=1) as qkv, \
                 tc.tile_pool(name="smalls", bufs=1) as sml:
                qT = qkv.tile([128, H, TOK], BF16)
                kT = qkv.tile([128, HK, EXT], BF16)
                Vb = qkv.tile([128, KT, HK * D], BF16)

                def bcast_row(row_f32r, width, tag):
                    """[1,width] F32R row -> list of (psum[128,w], j, w)."""
                    parts = []
                    for j in range(0, width, 512):
                        w = min(512, width - j)
                        p = bcps.tile([128, w], F32, tag="bc",
                                      name=f"bc_{tag}_{j}")
                        nc.tensor.matmul(p[:], oner[:],
                                         row_f32r[:, j:j + w],
                                         start=True, stop=True)
                        parts.append((p, j, w))
                    return parts

                def row_rsqrt_bcast(acc_parts, width, denom, tag):
                    """psum [1,*] sum-of-squares parts -> [128,*] psum
                    chunks with 1/sqrt(acc/denom + eps) bcast."""
                    srow = sml.tile([1, width], F32, tag="srow")
                    for (p, j, w) in acc_parts:
                        nc.scalar.activation(out=srow[:, j:j + w], in_=p[:],
                                             func=AF.Sqrt, bias=eps_s[:],
                                             scale=1.0 / denom)
                    rrow = sml.tile([1, width], F32R, tag="rrow")
                    with nc.allow_low_precision(reason="f32r intended"):
                        nc.vector.reciprocal(rrow[:], srow[:])
                    return bcast_row(rrow, width, tag)

                # ============ projections, two 512-token halves (own first)
                with tc.tile_pool(name="hp", bufs=1) as hp, \
                     tc.tile_pool(name="nrm", bufs=1) as nrm, \
                     tc.tile_pool(name="xs", bufs=3) as xs, \
                     tc.tile_pool(name="sqp", bufs=3) as sqp, \
                     tc.tile_pool(name="ws", bufs=8) as ws, \
                     tc.tile_pool(name="accP", bufs=4, space="PSUM") as accP:
                    # rope tables
                    cq_s = nrm.tile([RD, TOK], F32)
                    nc.sync.dma_start(cq_s[:], cos_q[:])
                    sq_s = nrm.tile([RD, TOK], F32)
                    nc.sync.dma_start(sq_s[:], sinS_q[:])
                    ck_s = nrm.tile([RD, EXT], F32)
                    nc.sync.dma_start(ck_s[:], cos_k[:])
                    sk_s = nrm.tile([RD, EXT], F32)
                    nc.sync.dma_start(sk_s[:], sinS_k[:])

                    def rope(t3, nh, width, cos_t, sinS_t):
                        c3 = cos_t[:].rearrange(
                            "p (g t) -> p g t",
                            g=1).broadcast_to([RD, nh, width])
                        s3 = sinS_t[:].rearrange(
                            "p (g t) -> p g t",
                            g=1).broadcast_to([RD, nh, width])
                        # partner halves moved onto matching partitions
                        qsw = nrm.tile([RD, nh, width], BF16, tag="rsw")
                        nc.sync.dma_start(qsw[0:32], t3[32:64])
                        nc.sync.dma_start(qsw[32:64], t3[0:32])
                        t1 = nrm.tile([RD, nh, width], BF16, tag="rt1")
                        nc.vector.tensor_mul(t1[:], t3[0:RD], c3)
                        nc.vector.tensor_mul(qsw[:], qsw[:], s3)
                        nc.vector.tensor_add(t3[0:RD], t1[:], qsw[:])

                    vT = hp.tile([128, HK, EXT], BF16, tag="vT")
                    for half in (1, 0):
                        c0 = half * 512
                        acc = rps.tile([1, 512], F32, tag="row")
                        for i in range(NM):
                            xt = xs.tile([128, 512], F32R, tag="xt")
                            nc.sync.dma_start(
                                xt[:], xT[i * 128:(i + 1) * 128, c0:c0 + 512])
                            sq = sqp.tile([128, 512], BF16, tag="sq")
                            nc.vector.tensor_mul(sq[:], xt[:], xt[:])
                            nc.tensor.matmul(acc[:], oneb[:], sq[:],
                                             start=(i == 0), stop=(i == NM - 1))
                        s1b = row_rsqrt_bcast([(acc, 0, 512)], 512, HID,
                                               "l1")[0][0]
                        hTh = hp.tile([128, NM, 512], BF16, tag="h")
                        for i in range(NM):
                            xt = xs.tile([128, 512], F32R, tag="xt")
                            nc.sync.dma_start(
                                xt[:], xT[i * 128:(i + 1) * 128, c0:c0 + 512])
                            nc.vector.tensor_mul(hTh[:, i, :], xt[:], s1b[:])

                        if half == 1:
                            # Q projection (own tokens only), 4-head groups
                            for mg in range(4):
                                pq = [accP.tile([128, 512], F32, tag="acc",
                                                 name=f"pq{j}")
                                      for j in range(4)]
                                for i in range(NM):
                                    wq_t = ws.tile([128, 512], BF16, tag="wq")
                                    nc.gpsimd.dma_start(
                                        wq_t[:],
                                        wqT[i * 128:(i + 1) * 128,
                                            mg * 512:(mg + 1) * 512])
                                    for j in range(4):
                                        nc.tensor.matmul(
                                            pq[j][:],
                                            wq_t[:, j * 128:(j + 1) * 128],
                                            hTh[:, i, :],
                                            start=(i == 0), stop=(i == NM - 1))
                                for j in range(4):
                                    nc.vector.tensor_copy(
                                        qT[:, mg * 4 + j, :], pq[j][:])

                        # K / V for this half (i-outer, 4 kv heads inner)
                        pk = [accP.tile([128, 512], F32, tag="acc",
                                         name=f"pk{g}")
                              for g in range(4)]
                        for i in range(NM):
                            wk_t = ws.tile([128, 512], BF16, tag="wk")
                            nc.gpsimd.dma_start(
                                wk_t[:], wkT[i * 128:(i + 1) * 128, :])
                            for g in range(HK):
                                nc.tensor.matmul(
                                    pk[g][:], wk_t[:, g * 128:(g + 1) * 128],
                                    hTh[:, i, :],
                                    start=(i == 0), stop=(i == NM - 1))
                        for g in range(HK):
                            nc.vector.tensor_copy(kT[:, g, c0:c0 + 512],
                                                  pk[g][:])
                        pv = [accP.tile([128, 512], F32, tag="acc",
                                         name=f"pv{g}")
                              for g in range(4)]
                        for i in range(NM):
                            wv_t = ws.tile([128, 512], BF16, tag="wv")
                            nc.gpsimd.dma_start(
                                wv_t[:], wvT[i * 128:(i + 1) * 128, :])
                            for g in range(HK):
                                nc.tensor.matmul(
                                    pv[g][:], wv_t[:, g * 128:(g + 1) * 128],
                                    hTh[:, i, :],
                                    start=(i == 0), stop=(i == NM - 1))
                        for g in range(HK):
                            nc.vector.tensor_copy(vT[:, g, c0:c0 + 512],
                                                  pv[g][:])

                        if half == 1:
                            # fused q RMSNorm + rope; overlaps half-0 K/V
                            accq = rps.tile([1, 512], F32, tag="row")
                            sqq = nrm.tile([128, TOK], BF16, tag="nsq")
                            for h in range(H):
                                nc.vector.tensor_mul(sqq[:], qT[:, h, :],
                                                     qT[:, h, :])
                                nc.tensor.matmul(accq[:], oneb[:], sqq[:],
                                                 start=(h == 0),
                                                 stop=(h == H - 1))
                            cqb = row_rsqrt_bcast([(accq, 0, 512)], TOK,
                                                  H * D, "qn")[0][0]
                            cqb3 = cqb[:].rearrange(
                                "p (g t) -> p g t",
                                g=1).broadcast_to([128, H, TOK])
                            nc.vector.tensor_mul(qT[:], qT[:], cqb3)
                            for h in range(H):
                                nc.vector.tensor_scalar_mul(
                                    qT[:, h, :], qT[:, h, :],
                                    qnw_s[:, h:h + 1])
                            rope(qT[:, 0:8, :], 8, TOK, cq_s, sq_s)
                            rope(qT[:, 8:16, :], 8, TOK, cq_s, sq_s)

                    # resident MLP weight loads (gpsimd queue, after qkv w)
                    for i in range(NM):
                        nc.gpsimd.dma_start(
                            w1s[:, i, :], w1T[i * 128:(i + 1) * 128, :])
                    for i in range(NM):
                        nc.gpsimd.dma_start(
                            w3s[:, i, :], w3T[i * 128:(i + 1) * 128, :])

                    # k RMSNorm + rope
                    acck_lo = rps.tile([1, 512], F32, tag="row")
                    acck_hi = rps.tile([1, 512], F32, tag="row")
                    sqk = nrm.tile([128, EXT], BF16, tag="nsqk")
                    for g in range(HK):
                        nc.vector.tensor_mul(sqk[:], kT[:, g, :], kT[:, g, :])
                        nc.tensor.matmul(acck_lo[:], oneb[:], sqk[:, 0:512],
                                         start=(g == 0), stop=(g == HK - 1))
                        nc.tensor.matmul(acck_hi[:], oneb[:], sqk[:, 512:1024],
                                         start=(g == 0), stop=(g == HK - 1))
                    ckb_parts = row_rsqrt_bcast(
                        [(acck_lo, 0, 512), (acck_hi, 512, 512)],
                        EXT, HK * D, "kn")
                    for (pck, j, w) in ckb_parts:
                        v = pck[:].rearrange(
                            "p (g t) -> p g t",
                            g=1).broadcast_to([128, HK, w])
                        nc.vector.tensor_mul(kT[:, :, j:j + w],
                                             kT[:, :, j:j + w], v)
                    for g in range(HK):
                        nc.vector.tensor_scalar_mul(
                            kT[:, g, :], kT[:, g, :], knw_s[:, g:g + 1])
                    rope(kT[:], HK, EXT, ck_s, sk_s)

                    # transpose vT -> token-major bf16 Vb
                    for kt in range(KT):
                        for g in range(HK):
                            pt = accP.tile([128, 128], BF16, tag="acc")
                            nc.tensor.transpose(
                                pt[:], vT[:, g, kt * 128:(kt + 1) * 128],
                                identB[:])
                            nc.vector.tensor_copy(
                                Vb[:, kt, g * 128:(g + 1) * 128], pt[:])

                # ============ sliding-window attention
                with tc.tile_pool(name="attn", bufs=1) as ap, \
                     tc.tile_pool(name="es", bufs=12) as es, \
                     tc.tile_pool(name="bps", bufs=4, space="PSUM") as bps:
                    attnT = ap.tile([128, H, TOK], BF16)
                    for h in range(H):
                        g = h // (H // HK)
                        exps = []
                        for kt in range(KT):
                            ps = bps.tile([128, 512], F32, tag="big")
                            nc.tensor.matmul(
                                ps[:], kT[:, g, kt * 128:(kt + 1) * 128],
                                qT[:, h, :], start=True, stop=True)
                            e = es.tile([128, 512], BF16, tag="e")
                            nc.scalar.activation(
                                out=e[:], in_=ps[:], func=AF.Exp,
                                bias=halo_s[:, kt:kt + 1], scale=SCALE)
                            nc.vector.tensor_mul(
                                e[:], e[:],
                                band_s[:, 896 - 128 * kt:1408 - 128 * kt])
                            exps.append(e)
                        den = rps.tile([1, 512], F32, tag="row")
                        for kt in range(KT):
                            nc.tensor.matmul(den[:], oneb[:], exps[kt][:],
                                             start=(kt == 0),
                                             stop=(kt == KT - 1))
                        dr = sml.tile([1, 512], F32, tag="dr")
                        nc.vector.tensor_copy(dr[:], den[:])
                        drr = sml.tile([1, 512], F32R, tag="drr")
                        with nc.allow_low_precision(reason="f32r intended"):
                            nc.vector.reciprocal(drr[:], dr[:])
                        rb = bcast_row(drr, 512, "rden")[0][0]
                        rbs = sml.tile([128, 512], F32R, tag="rbs")
                        nc.vector.tensor_copy(rbs[:], rb[:])
                        po = bps.tile([128, 512], F32, tag="big")
                        for kt in range(KT):
                            nc.tensor.matmul(
                                po[:],
                                Vb[:, kt, g * 128:(g + 1) * 128],
                                exps[kt][:], start=(kt == 0),
                                stop=(kt == KT - 1))
                        nc.vector.tensor_mul(attnT[:, h, :], po[:], rbs[:])

                    # ============ o_proj + residual + ln2
                    with tc.tile_pool(name="x2", bufs=1) as x2p, \
                         tc.tile_pool(name="wos", bufs=8) as wos, \
                         tc.tile_pool(name="xs2", bufs=3) as xs2:
                        x2T = x2p.tile([128, NM, TOK], BF16)
                        acc2 = rps.tile([1, 512], F32, tag="row")
                        for mg in range(4):
                            px = [bps.tile([128, 512], F32, tag="big",
                                            name=f"px{j}")
                                  for j in range(4)]
                            for i in range(NM):
                                wo_t = wos.tile([128, 512], BF16, tag="wo")
                                nc.gpsimd.dma_start(
                                    wo_t[:], woT[i * 128:(i + 1) * 128,
                                                 mg * 512:(mg + 1) * 512])
                                for j in range(4):
                                    nc.tensor.matmul(
                                        px[j][:],
                                        wo_t[:, j * 128:(j + 1) * 128],
                                        attnT[:, i, :],
                                        start=(i == 0), stop=(i == NM - 1))
                            for j in range(4):
                                m = mg * 4 + j
                                xo = xs2.tile([128, TOK], F32R, tag="xo")
                                nc.sync.dma_start(
                                    xo[:], xT[m * 128:(m + 1) * 128, 512:1024])
                                nc.vector.tensor_add(x2T[:, m, :], px[j][:],
                                                     xo[:])
                                sq2 = xs2.tile([128, TOK], BF16, tag="sq2")
                                nc.vector.tensor_mul(sq2[:], x2T[:, m, :],
                                                     x2T[:, m, :])
                                nc.tensor.matmul(acc2[:], oneb[:], sq2[:],
                                                 start=(m == 0),
                                                 stop=(m == NM - 1))

                        # ln2 -> h2 -> AG inputs (first, so AG starts early)
                        s2b = row_rsqrt_bcast([(acc2, 0, 512)], TOK, HID,
                                                "l2")[0][0]
                        for m in range(NM):
                            h2t = xs2.tile([128, TOK], BF16, tag="h2t")
                            nc.vector.tensor_mul(h2t[:], x2T[:, m, :], s2b[:])
                            for q in range(NQ):
                                eng = nc.sync if q < 2 else nc.scalar
                                eng.dma_start(
                                    ag_in[q][m * 128:(m + 1) * 128, :],
                                    h2t[:, q * QW:(q + 1) * QW])

                        for q in range(NQ):
                            nc.gpsimd.collective_compute(
                                "AllGather", mybir.AluOpType.bypass,
                                replica_groups=[list(range(NCORES))],
                                ins=[ag_in[q].opt()], outs=[ag_out[q].opt()],
                            )

                        # x2 token-major -> DRAM (overlaps the AllGather)
                        for tt in range(4):
                            for grp in range(4):
                                ts = xs2.tile([128, 512], BF16, tag="x2t")
                                for j in range(4):
                                    m = grp * 4 + j
                                    pt = bps.tile([128, 128], BF16, tag="big")
                                    nc.tensor.transpose(
                                        pt[:],
                                        x2T[:, m, tt * 128:(tt + 1) * 128],
                                        identB[:])
                                    nc.vector.tensor_copy(
                                        ts[:, j * 128:(j + 1) * 128], pt[:])
                                nc.gpsimd.dma_start(
                                    out[tt * 128:(tt + 1) * 128,
                                        grp * 512:(grp + 1) * 512], ts[:])

            # ============ TP MLP: chunks of 512 tokens (4 cores x 128 cols)
            with tc.tile_pool(name="gsp", bufs=1) as gsp, \
                 tc.tile_pool(name="mh", bufs=2) as mh, \
                 tc.tile_pool(name="silp", bufs=2) as silp, \
                 tc.tile_pool(name="w2s", bufs=2) as w2sp, \
                 tc.tile_pool(name="po", bufs=8) as pop, \
                 tc.tile_pool(name="mps", bufs=4, space="PSUM") as mps:
                gs = gsp.tile([128, NMI, NCORES * TOK], BF16)
                for q in range(NQ):
                    for quad in range(2):
                        h2c = mh.tile([128, NM, 512], BF16, tag="h2")
                        for i in range(NM):
                            src = ag_out[q][quad * 4:(quad + 1) * 4,
                                            i * 128:(i + 1) * 128, :]
                            nc.sync.dma_start(
                                h2c[:, i, :].rearrange(
                                    "p (c w) -> p c w", c=4),
                                src.rearrange("c p w -> p c w"))
                        for m in range(NMI):
                            a = mps.tile([128, 512], F32, tag="m")
                            for i in range(NM):
                                nc.tensor.matmul(
                                    a[:], w1s[:, i, m * 128:(m + 1) * 128],
                                    h2c[:, i, :],
                                    start=(i == 0), stop=(i == NM - 1))
                            sil = silp.tile([128, 512], BF16, tag="sil")
                            nc.scalar.activation(out=sil[:], in_=a[:],
                                                 func=AF.Silu)
                            b = mps.tile([128, 512], F32, tag="m")
                            for i in range(NM):
                                nc.tensor.matmul(
                                    b[:], w3s[:, i, m * 128:(m + 1) * 128],
                                    h2c[:, i, :],
                                    start=(i == 0), stop=(i == NM - 1))
                            dst = gs[:, m,
                                     quad * 4 * TOK:(quad * 4 + 4) * TOK]
                            dst3 = dst.rearrange("p (c w) -> p c w", c=4)
                            nc.vector.tensor_mul(
                                dst3[:, :, q * QW:(q + 1) * QW],
                                sil[:].rearrange("p (c w) -> p c w", c=4),
                                b[:].rearrange("p (c w) -> p c w", c=4))

                # w2 by output-column slice; pipelined ReduceScatter + tail
                w2tiles = {}

                def load_w2(hs):
                    t = w2sp.tile([128, NMI, 512], BF16, tag="w2c",
                                  name=f"w2c{hs}")
                    for m in range(NMI):
                        nc.gpsimd.dma_start(
                            t[:, m, :], w2T[m * 128:(m + 1) * 128,
                                            hs * 512:(hs + 1) * 512])
                    w2tiles[hs] = t

                load_w2(0)
                load_w2(1)
                for hs in range(4):
                    w2c = w2tiles[hs]
                    for tt in range(NCORES * TOK // 128):
                        op_ = mps.tile([128, 512], F32, tag="m")
                        for m in range(NMI):
                            nc.tensor.matmul(
                                op_[:], gs[:, m, tt * 128:(tt + 1) * 128],
                                w2c[:, m, :],
                                start=(m == 0), stop=(m == NMI - 1))
                        pb = pop.tile([128, 512], BF16, tag="pb")
                        nc.vector.tensor_copy(pb[:], op_[:])
                        nc.sync.dma_start(
                            rs_in[hs][tt * 128:(tt + 1) * 128, :], pb[:])
                    if hs + 2 < 4:
                        load_w2(hs + 2)
                    nc.gpsimd.collective_compute(
                        "ReduceScatter", mybir.AluOpType.add,
                        replica_groups=[list(range(NCORES))],
                        ins=[rs_in[hs].opt()], outs=[rs_out[hs].opt()],
                    )

                with tc.tile_pool(name="tail", bufs=4) as tp:
                    for hs in range(4):
                        for tt in range(4):
                            rsf = tp.tile([128, 512], F32, tag="rsf")
                            nc.gpsimd.dma_start(
                                rsf[:], rs_out[hs][tt * 128:(tt + 1) * 128, :])
                            nc.gpsimd.dma_start(
                                out[tt * 128:(tt + 1) * 128,
                                    hs * 512:(hs + 1) * 512], rsf[:],
                                accum_op=mybir.AluOpType.add)

    return nc


# ---------------------------------------------------------------- host side
def _rope_tables(pos):
    inv = 1.0 / (THETA ** (np.arange(0, RD, 2, dtype=np.float32) / RD))
    f = pos[:, None].astype(np.float32) * inv[None, :]
    emb = np.concatenate([f, f], axis=-1)          # [T, RD]
    cos = np.ascontiguousarray(np.cos(emb).T)      # [RD, T]
    sin = np.sin(emb).T
    sinS = sin.copy()
    sinS[0:32] = -sin[0:32]
    return cos.astype(np.float32), np.ascontiguousarray(sinS).astype(np.float32)


def _band_mask():
    import ml_dtypes
    p = np.arange(128)[:, None]
    u = np.arange(1408)[None, :]
    m = ((u >= p + 384) & (u <= p + 896)).astype(np.float32)
    return m.astype(ml_dtypes.bfloat16)


def _prepare_in_maps(hidden_states, wq, wk, wv, wo, q_norm_w, k_norm_w,
                     ln1_w, ln2_w, w1, w2, w3):
    import ml_dtypes
    xf = np.ascontiguousarray(hidden_states.reshape(B * S, HID))
    # fold ln1_w into wq/wk/wv rows, ln2_w into w1/w3 rows
    wqTn = np.ascontiguousarray(wq.T * ln1_w[:, None])
    wkTn = np.ascontiguousarray(wk.T * ln1_w[:, None])
    wvTn = np.ascontiguousarray(wv.T * ln1_w[:, None])
    woTn = np.ascontiguousarray(wo.T)
    w1Tn = np.ascontiguousarray(w1.T * ln2_w[:, None])
    w3Tn = np.ascontiguousarray(w3.T * ln2_w[:, None])
    w2Tn = np.ascontiguousarray(w2.T)
    qnc = np.ascontiguousarray(q_norm_w.reshape(16, 128).T)
    knc = np.ascontiguousarray(k_norm_w.reshape(4, 128).T)
    band = _band_mask()
    ones_r = np.ones((1, 128), np.float32)
    ones_b = np.ones((128, 1), ml_dtypes.bfloat16)

    in_maps = []
    for c in range(NCORES):
        t0 = c * TOK
        bidx = t0 // S
        s0 = t0 % S
        xe = np.zeros((EXT, HID), np.float32)
        lo = s0 - WIN
        if lo >= 0:
            xe[:] = xf[bidx * S + lo: bidx * S + s0 + TOK]
            halo_valid = True
        else:
            xe[WIN:] = xf[bidx * S + s0: bidx * S + s0 + TOK]
            halo_valid = False
        xTc = np.ascontiguousarray(xe.T)

        qpos = np.arange(s0, s0 + TOK)
        kpos = np.arange(s0 - WIN, s0 + TOK)
        cq, sq_ = _rope_tables(qpos)
        ck, sk_ = _rope_tables(np.maximum(kpos, 0))
        halo_bias = np.zeros(EXT, np.float32)
        if not halo_valid:
            halo_bias[0:WIN] = NEG
        haloc = np.ascontiguousarray(halo_bias.reshape(8, 128).T)

        in_maps.append({
            "xT": xTc,
            "wqT": wqTn, "wkT": wkTn, "wvT": wvTn, "woT": woTn,
            "w1T": np.ascontiguousarray(w1Tn[:, c * IMC:(c + 1) * IMC]),
            "w3T": np.ascontiguousarray(w3Tn[:, c * IMC:(c + 1) * IMC]),
            "w2T": np.ascontiguousarray(w2Tn[c * IMC:(c + 1) * IMC, :]),
            "qnw": qnc, "knw": knc,
            "cos_q": cq, "sinS_q": sq_, "cos_k": ck, "sinS_k": sk_,
            "halo": haloc, "band": band,
            "ones_r": ones_r, "ones_b": ones_b,
        })
    return in_maps


_NC = None


def _get_nc():
    global _NC
    if _NC is None:
        _register_ntff_hook()
        _NC = build_nc()
    return _NC


def run(in_maps, trace=False):
    from concourse.bass_utils import run_bass_kernel_spmd
    nc = _get_nc()
    return run_bass_kernel_spmd(nc, in_maps, core_ids=list(range(NCORES)),
                                trace=trace)


def kernel(**inputs):
    arrs = {k: np.asarray(v, dtype=np.float32) for k, v in inputs.items()}
    in_maps = _prepare_in_maps(
        arrs["hidden_states"], arrs["wq"], arrs["wk"], arrs["wv"], arrs["wo"],
        arrs["q_norm_w"], arrs["k_norm_w"], arrs["ln1_w"], arrs["ln2_w"],
        arrs["w1"], arrs["w2"], arrs["w3"])
    res = run(in_maps, trace=False)
    full = np.empty((B * S, HID), np.float32)
    for c in range(NCORES):
        full[c * TOK:(c + 1) * TOK] = res.results[c]["out"]
    return full.reshape(B, S, HID)


# revision 15
# speedup vs baseline: 1.1201x; 1.0106x over previous
"""MiniMax-M2 decoder layer on 8 trn2 NeuronCores.

Sharding: sequence-sharded attention (each core owns 512 tokens of the
flattened (B*S)=4096 token stream and recomputes the 512-token KV halo
locally -> no collectives in the attention block), tensor-parallel MLP
(IM=8192 sharded 1024/core; AllGather of the ln2-normed activations,
ReduceScatter of the w2 partial sums in bf16).

v2 restructure vs baseline:
- no DRAM bf16 pre-cast phase: weights stream via gpsimd casting DMAs
  (f32 DRAM -> bf16 SBUF in flight), in big [128,512]+ tiles.
- w1/w3 SBUF-resident bf16, loaded once during the attention phase.
- ln1_w/ln2_w folded into wq/wk/wv/w1/w3 host-side.
- AllGather split 4-ways along own-token columns, outputs in Shared
  DRAM; MLP chunks start as soon as the first slice lands.
- MLP keeps all gate activations (gs) in SBUF; w2 runs hs-slice-major
  with a pipelined 4-way ReduceScatter + per-slice residual tail.
- x2 residual stream in bf16 (rel-err budget allows).

Self-contained: includes the BIR wait-splitting fix this container's
walrus build needs (1 semaphore wait per instruction max).
"""

import json
import sys
import types

import numpy as np

import concourse.bass as bass
import concourse.mybir as mybir
import concourse.tile as tile
from concourse.masks import make_identity

# ---------------------------------------------------------------- constants
B, S, HID = 2, 2048, 2048
H, HK, D = 16, 4, 128
RD = 64
IM = 8192
WIN = 512
EPS = 1e-6
THETA = 10000.0
SCALE = D ** -0.5

NCORES = 8
TOK = 512              # own tokens per core
EXT = 1024             # halo + own
IMC = IM // NCORES     # 1024 im rows per core
NEG = -1e9

F32 = mybir.dt.float32
F32R = mybir.dt.float32r
BF16 = mybir.dt.bfloat16
AF = mybir.ActivationFunctionType

KT = 8                 # 128-wide key tiles over EXT
NM = HID // 128        # 16 hid tiles
NMI = IMC // 128       # 8 im tiles per core
NQ = 4                 # AllGather token-column split
QW = TOK // NQ         # 128 columns per AG slice

# ------------------------------------------------------- walrus wait-split fix
MAX_WAITS = 1


def _split_excess_waits(bir_bytes: bytes) -> bytes:
    m = json.loads(bir_bytes)
    ctr = [0]

    def fix_insts(insts):
        out = []
        for ins in insts:
            si = ins.get("sync_info")
            ow = (si or {}).get("on_wait") or []
            if len(ow) > MAX_WAITS:
                eng = ins["engine"]
                keep = ow[-MAX_WAITS:]
                excess = ow[:-MAX_WAITS]
                ins["sync_info"]["on_wait"] = keep
                for i in range(0, len(excess), MAX_WAITS):
                    ctr[0] += 1
                    out.append({
                        "debug": ins.get("debug", 0),
                        "engine": eng,
                        "ins": [],
                        "name": f"I-waitfix-{ctr[0]}",
                        "opcode": "NoOp",
                        "outs": [],
                        "sync_info": {"on_update": [],
                                      "on_wait": excess[i:i + MAX_WAITS]},
                        "text_hint": "waitfix",
                    })
            out.append(ins)
        return out

    def walk(o):
        if isinstance(o, dict):
            if isinstance(o.get("instructions"), list):
                o["instructions"] = fix_insts(o["instructions"])
            for v in o.values():
                walk(v)
        elif isinstance(o, list):
            for v in o:
                walk(v)

    walk(m)
    return json.dumps(m).encode()


class _BassFixed(bass.Bass):
    def to_json_bytes(self) -> bytes:
        return _split_excess_waits(super().to_json_bytes())


def _register_ntff_hook():
    """Provide antenv.axon_hooks (missing in this image) so trace=True works."""
    if "antenv.axon_hooks" in sys.modules:
        return
    try:
        import trn_agent_boot.trn_boot as tb
    except ImportError:
        return
    mod = types.ModuleType("antenv.axon_hooks")
    holder = [None]
    mod.set_axon_ntff_profile_hook = lambda h: holder.__setitem__(0, h)
    mod.get_axon_ntff_profile_hook = lambda: holder[0]
    sys.modules["antenv.axon_hooks"] = mod
    try:
        mod.set_axon_ntff_profile_hook(
            tb._ntff_profile_via_ctypes("/opt/axon/libaxon_pjrt.so"))
    except Exception:
        pass


# ---------------------------------------------------------------- the program
def build_nc():
    nc = _BassFixed(num_devices=NCORES, target_bir_lowering=False)

    xT = nc.dram_tensor("xT", [HID, EXT], F32R, kind="ExternalInput")
    wqT = nc.dram_tensor("wqT", [HID, H * D], F32R, kind="ExternalInput")
    wkT = nc.dram_tensor("wkT", [HID, HK * D], F32R, kind="ExternalInput")
    wvT = nc.dram_tensor("wvT", [HID, HK * D], F32R, kind="ExternalInput")
    woT = nc.dram_tensor("woT", [H * D, HID], F32R, kind="ExternalInput")
    w1T = nc.dram_tensor("w1T", [HID, IMC], F32R, kind="ExternalInput")
    w3T = nc.dram_tensor("w3T", [HID, IMC], F32R, kind="ExternalInput")
    w2T = nc.dram_tensor("w2T", [IMC, HID], F32R, kind="ExternalInput")
    qnw = nc.dram_tensor("qnw", [128, 16], F32, kind="ExternalInput")
    knw = nc.dram_tensor("knw", [128, 4], F32, kind="ExternalInput")
    cos_q = nc.dram_tensor("cos_q", [RD, TOK], F32, kind="ExternalInput")
    sinS_q = nc.dram_tensor("sinS_q", [RD, TOK], F32, kind="ExternalInput")
    cos_k = nc.dram_tensor("cos_k", [RD, EXT], F32, kind="ExternalInput")
    sinS_k = nc.dram_tensor("sinS_k", [RD, EXT], F32, kind="ExternalInput")
    halo = nc.dram_tensor("halo", [128, 8], F32, kind="ExternalInput")
    band = nc.dram_tensor("band", [128, 1408], BF16, kind="ExternalInput")
    ones_r = nc.dram_tensor("ones_r", [1, 128], F32R, kind="ExternalInput")
    ones_b = nc.dram_tensor("ones_b", [128, 1], BF16, kind="ExternalInput")

    out = nc.dram_tensor("out", [TOK, HID], F32, kind="ExternalOutput")

    with tile.TileContext(nc) as tc:
        with tc.tile_pool(name="consts", bufs=1) as cst, \
             tc.tile_pool(name="res", bufs=1) as res, \
             tc.tile_pool(name="dram", bufs=1, space="DRAM") as dram, \
             tc.tile_pool(name="rowps", bufs=3, space="PSUM") as rps, \
             tc.tile_pool(name="bcps", bufs=1, space="PSUM") as bcps:

            # ---------------- constants
            oner = cst.tile([1, 128], F32R)
            nc.sync.dma_start(oner[:], ones_r[:])
            oneb = cst.tile([128, 1], BF16)
            nc.sync.dma_start(oneb[:], ones_b[:])
            qnw_s = cst.tile([128, 16], F32)
            nc.sync.dma_start(qnw_s[:], qnw[:])
            knw_s = cst.tile([128, 4], F32)
            nc.sync.dma_start(knw_s[:], knw[:])
            halo_s = cst.tile([128, 8], F32)
            nc.sync.dma_start(halo_s[:], halo[:])
            band_s = cst.tile([128, 1408], BF16)
            nc.sync.dma_start(band_s[:], band[:])
            eps_s = cst.tile([1, 1], F32)
            nc.vector.memset(eps_s[:], EPS)
            identF = cst.tile([128, 128], F32)
            make_identity(nc, identF[:])
            identB = cst.tile([128, 128], BF16)
            nc.vector.tensor_copy(identB[:], identF[:])

            # tiny warm-up collective: absorbs the one-time CC entry
            # barrier (~40us) under the projection phase
            dummy_i = dram.tile([128, 16], BF16, name="dummy_i")
            dummy_o = dram.tile([NCORES, 128, 16], BF16,
                                addr_space="Shared", name="dummy_o")
            dmv = cst.tile([128, 16], BF16)
            nc.vector.memset(dmv[:], 0.0)
            nc.sync.dma_start(dummy_i[:], dmv[:])
            nc.gpsimd.collective_compute(
                "AllGather", mybir.AluOpType.bypass,
                replica_groups=[list(range(NCORES))],
                ins=[dummy_i.opt()], outs=[dummy_o.opt()],
            )

            # resident MLP weights (loads issued after QKV weight streaming)
            w1s = res.tile([128, NM, IMC], BF16)
            w3s = res.tile([128, NM, IMC], BF16)

            # internal DRAM
            ag_in = [dram.tile([HID, QW], BF16, name=f"ag_in{q}")
                     for q in range(NQ)]
            ag_out = [dram.tile([NCORES, HID, QW], BF16, addr_space="Shared",
                                name=f"ag_out{q}") for q in range(NQ)]
            rs_in = [dram.tile([NCORES * TOK, 512], BF16, name=f"rs_in{h}")
                     for h in range(4)]
            rs_out = [dram.tile([TOK, 512], BF16, name=f"rs_out{h}")
                      for h in range(4)]

            # ======== attention block scope
            with tc.tile_pool(name="qkv", bufs=1) as qkv, \
                 tc.tile_pool(name="smalls", bufs=1) as sml:
                qT = qkv.tile([128, H, TOK], BF16)
                kT = qkv.tile([128, HK, EXT], BF16)
                Vb = qkv.tile([128, KT, HK * D], BF16)

                def bcast_row(row_f32r, width, tag):
                    """[1,width] F32R row -> list of (psum[128,w], j, w)."""
                    parts = []
                    for j in range(0, width, 512):
                        w = min(512, width - j)
                        p = bcps.tile([128, w], F32, tag="bc",
                                      name=f"bc_{tag}_{j}")
                        nc.tensor.matmul(p[:], oner[:],
                                         row_f32r[:, j:j + w],
                                         start=True, stop=True)
                        parts.append((p, j, w))
                    return parts

                def row_rsqrt_bcast(acc_parts, width, denom, tag):
                    """psum [1,*] sum-of-squares parts -> [128,*] psum
                    chunks with 1/sqrt(acc/denom + eps) bcast."""
                    srow = sml.tile([1, width], F32, tag="srow")
                    for (p, j, w) in acc_parts:
                        nc.scalar.activation(out=srow[:, j:j + w], in_=p[:],
                                             func=AF.Sqrt, bias=eps_s[:],
                                             scale=1.0 / denom)
                    rrow = sml.tile([1, width], F32R, tag="rrow")
                    with nc.allow_low_precision(reason="f32r intended"):
                        nc.vector.reciprocal(rrow[:], srow[:])
                    return bcast_row(rrow, width, tag)

                # ============ projections, two 512-token halves (own first)
                with tc.tile_pool(name="hp", bufs=1) as hp, \
                     tc.tile_pool(name="nrm", bufs=1) as nrm, \
                     tc.tile_pool(name="xs", bufs=3) as xs, \
                     tc.tile_pool(name="sqp", bufs=3) as sqp, \
                     tc.tile_pool(name="ws", bufs=8) as ws, \
                     tc.tile_pool(name="accP", bufs=4, space="PSUM") as accP:
                    # rope tables
                    cq_s = nrm.tile([RD, TOK], F32)
                    nc.sync.dma_start(cq_s[:], cos_q[:])
                    sq_s = nrm.tile([RD, TOK], F32)
                    nc.sync.dma_start(sq_s[:], sinS_q[:])
                    ck_s = nrm.tile([RD, EXT], F32)
                    nc.sync.dma_start(ck_s[:], cos_k[:])
                    sk_s = nrm.tile([RD, EXT], F32)
                    nc.sync.dma_start(sk_s[:], sinS_k[:])

                    def rope(t3, nh, width, cos_t, sinS_t):
                        c3 = cos_t[:].rearrange(
                            "p (g t) -> p g t",
                            g=1).broadcast_to([RD, nh, width])
                        s3 = sinS_t[:].rearrange(
                            "p (g t) -> p g t",
                            g=1).broadcast_to([RD, nh, width])
                        # partner halves moved onto matching partitions
                        qsw = nrm.tile([RD, nh, width], BF16, tag="rsw")
                        nc.sync.dma_start(qsw[0:32], t3[32:64])
                        nc.sync.dma_start(qsw[32:64], t3[0:32])
                        t1 = nrm.tile([RD, nh, width], BF16, tag="rt1")
                        nc.vector.tensor_mul(t1[:], t3[0:RD], c3)
                        nc.vector.tensor_mul(qsw[:], qsw[:], s3)
                        nc.vector.tensor_add(t3[0:RD], t1[:], qsw[:])

                    vT = hp.tile([128, HK, EXT], BF16, tag="vT")
                    for half in (1, 0):
                        c0 = half * 512
                        acc = rps.tile([1, 512], F32, tag="row")
                        for i in range(NM):
                            xt = xs.tile([128, 512], F32R, tag="xt")
                            nc.sync.dma_start(
                                xt[:], xT[i * 128:(i + 1) * 128, c0:c0 + 512])
                            sq = sqp.tile([128, 512], BF16, tag="sq")
                            nc.vector.tensor_mul(sq[:], xt[:], xt[:])
                            nc.tensor.matmul(acc[:], oneb[:], sq[:],
                                             start=(i == 0), stop=(i == NM - 1))
                        s1b = row_rsqrt_bcast([(acc, 0, 512)], 512, HID,
                                               "l1")[0][0]
                        hTh = hp.tile([128, NM, 512], BF16, tag="h")
                        for i in range(NM):
                            xt = xs.tile([128, 512], F32R, tag="xt")
                            nc.sync.dma_start(
                                xt[:], xT[i * 128:(i + 1) * 128, c0:c0 + 512])
                            nc.vector.tensor_mul(hTh[:, i, :], xt[:], s1b[:])

                        if half == 1:
                            # Q projection (own tokens only), 4-head groups
                            for mg in range(4):
                                pq = [accP.tile([128, 512], F32, tag="acc",
                                                 name=f"pq{j}")
                                      for j in range(4)]
                                for i in range(NM):
                                    wq_t = ws.tile([128, 512], BF16, tag="wq")
                                    nc.gpsimd.dma_start(
                                        wq_t[:],
                                        wqT[i * 128:(i + 1) * 128,
                                            mg * 512:(mg + 1) * 512])
                                    for j in range(4):
                                        nc.tensor.matmul(
                                            pq[j][:],
                                            wq_t[:, j * 128:(j + 1) * 128],
                                            hTh[:, i, :],
                                            start=(i == 0), stop=(i == NM - 1))
                                for j in range(4):
                                    nc.scalar.activation(
                                        out=qT[:, mg * 4 + j, :],
                                        in_=pq[j][:], func=AF.Copy)

                        # K / V for this half (i-outer, 4 kv heads inner)
                        pk = [accP.tile([128, 512], F32, tag="acc",
                                         name=f"pk{g}")
                              for g in range(4)]
                        for i in range(NM):
                            wk_t = ws.tile([128, 512], BF16, tag="wk")
                            nc.gpsimd.dma_start(
                                wk_t[:], wkT[i * 128:(i + 1) * 128, :])
                            for g in range(HK):
                                nc.tensor.matmul(
                                    pk[g][:], wk_t[:, g * 128:(g + 1) * 128],
                                    hTh[:, i, :],
                                    start=(i == 0), stop=(i == NM - 1))
                        for g in range(HK):
                            nc.scalar.activation(out=kT[:, g, c0:c0 + 512],
                                                 in_=pk[g][:], func=AF.Copy)
                        pv = [accP.tile([128, 512], F32, tag="acc",
                                         name=f"pv{g}")
                              for g in range(4)]
                        for i in range(NM):
                            wv_t = ws.tile([128, 512], BF16, tag="wv")
                            nc.gpsimd.dma_start(
                                wv_t[:], wvT[i * 128:(i + 1) * 128, :])
                            for g in range(HK):
                                nc.tensor.matmul(
                                    pv[g][:], wv_t[:, g * 128:(g + 1) * 128],
                                    hTh[:, i, :],
                                    start=(i == 0), stop=(i == NM - 1))
                        for g in range(HK):
                            nc.scalar.activation(out=vT[:, g, c0:c0 + 512],
                                                 in_=pv[g][:], func=AF.Copy)


                    # fused q RMSNorm + rope (vector) overlaps half-0 K/V (PE)
                    accq = rps.tile([1, 512], F32, tag="row")
                    sqq = nrm.tile([128, TOK], BF16, tag="nsq")
                    for h in range(H):
                        nc.vector.tensor_mul(sqq[:], qT[:, h, :], qT[:, h, :])
                        nc.tensor.matmul(accq[:], oneb[:], sqq[:],
                                         start=(h == 0), stop=(h == H - 1))
                    cqb = row_rsqrt_bcast([(accq, 0, 512)], TOK,
                                          H * D, "qn")[0][0]
                    cqb3 = cqb[:].rearrange(
                        "p (g t) -> p g t", g=1).broadcast_to([128, H, TOK])
                    nc.vector.tensor_mul(qT[:], qT[:], cqb3)
                    for h in range(H):
                        nc.vector.tensor_scalar_mul(
                            qT[:, h, :], qT[:, h, :], qnw_s[:, h:h + 1])
                    rope(qT[:, 0:8, :], 8, TOK, cq_s, sq_s)
                    rope(qT[:, 8:16, :], 8, TOK, cq_s, sq_s)

                    # resident MLP weight loads (gpsimd queue, after qkv w)
                    for i in range(NM):
                        nc.gpsimd.dma_start(
                            w1s[:, i, :], w1T[i * 128:(i + 1) * 128, :])
                    for i in range(NM):
                        nc.gpsimd.dma_start(
                            w3s[:, i, :], w3T[i * 128:(i + 1) * 128, :])

                    # k RMSNorm + rope
                    acck_lo = rps.tile([1, 512], F32, tag="row")
                    acck_hi = rps.tile([1, 512], F32, tag="row")
                    sqk = nrm.tile([128, EXT], BF16, tag="nsqk")
                    for g in range(HK):
                        nc.vector.tensor_mul(sqk[:], kT[:, g, :], kT[:, g, :])
                        nc.tensor.matmul(acck_lo[:], oneb[:], sqk[:, 0:512],
                                         start=(g == 0), stop=(g == HK - 1))
                        nc.tensor.matmul(acck_hi[:], oneb[:], sqk[:, 512:1024],
                                         start=(g == 0), stop=(g == HK - 1))
                    ckb_parts = row_rsqrt_bcast(
                        [(acck_lo, 0, 512), (acck_hi, 512, 512)],
                        EXT, HK * D, "kn")
                    for (pck, j, w) in ckb_parts:
                        v = pck[:].rearrange(
                            "p (g t) -> p g t",
                            g=1).broadcast_to([128, HK, w])
                        nc.vector.tensor_mul(kT[:, :, j:j + w],
                                             kT[:, :, j:j + w], v)
                    for g in range(HK):
                        nc.vector.tensor_scalar_mul(
                            kT[:, g, :], kT[:, g, :], knw_s[:, g:g + 1])
                    rope(kT[:], HK, EXT, ck_s, sk_s)

                    # transpose vT -> token-major bf16 Vb
                    for kt in range(KT):
                        for g in range(HK):
                            pt = accP.tile([128, 128], BF16, tag="acc")
                            nc.tensor.transpose(
                                pt[:], vT[:, g, kt * 128:(kt + 1) * 128],
                                identB[:])
                            nc.vector.tensor_copy(
                                Vb[:, kt, g * 128:(g + 1) * 128], pt[:])

                # ============ sliding-window attention
                with tc.tile_pool(name="attn", bufs=1) as ap, \
                     tc.tile_pool(name="es", bufs=12) as es, \
                     tc.tile_pool(name="bps", bufs=4, space="PSUM") as bps:
                    attnT = ap.tile([128, H, TOK], BF16)
                    for h in range(H):
                        g = h // (H // HK)
                        exps = []
                        for kt in range(KT):
                            ps = bps.tile([128, 512], F32, tag="big")
                            nc.tensor.matmul(
                                ps[:], kT[:, g, kt * 128:(kt + 1) * 128],
                                qT[:, h, :], start=True, stop=True)
                            e = es.tile([128, 512], BF16, tag="e")
                            nc.scalar.activation(
                                out=e[:], in_=ps[:], func=AF.Exp,
                                bias=halo_s[:, kt:kt + 1], scale=SCALE)
                            nc.vector.tensor_mul(
                                e[:], e[:],
                                band_s[:, 896 - 128 * kt:1408 - 128 * kt])
                            exps.append(e)
                        den = rps.tile([1, 512], F32, tag="row")
                        for kt in range(KT):
                            nc.tensor.matmul(den[:], oneb[:], exps[kt][:],
                                             start=(kt == 0),
                                             stop=(kt == KT - 1))
                        dr = sml.tile([1, 512], F32, tag="dr")
                        nc.vector.tensor_copy(dr[:], den[:])
                        drr = sml.tile([1, 512], F32R, tag="drr")
                        with nc.allow_low_precision(reason="f32r intended"):
                            nc.vector.reciprocal(drr[:], dr[:])
                        rb = bcast_row(drr, 512, "rden")[0][0]
                        rbs = sml.tile([128, 512], F32R, tag="rbs")
                        nc.vector.tensor_copy(rbs[:], rb[:])
                        po = bps.tile([128, 512], F32, tag="big")
                        for kt in range(KT):
                            nc.tensor.matmul(
                                po[:],
                                Vb[:, kt, g * 128:(g + 1) * 128],
                                exps[kt][:], start=(kt == 0),
                                stop=(kt == KT - 1))
                        nc.vector.tensor_mul(attnT[:, h, :], po[:], rbs[:])

                    # ============ o_proj + residual + ln2
                    with tc.tile_pool(name="x2", bufs=1) as x2p, \
                         tc.tile_pool(name="wos", bufs=8) as wos, \
                         tc.tile_pool(name="xs2", bufs=3) as xs2:
                        x2T = x2p.tile([128, NM, TOK], BF16)
                        acc2 = rps.tile([1, 512], F32, tag="row")
                        for mg in range(4):
                            px = [bps.tile([128, 512], F32, tag="big",
                                            name=f"px{j}")
                                  for j in range(4)]
                            for i in range(NM):
                                wo_t = wos.tile([128, 512], BF16, tag="wo")
                                nc.gpsimd.dma_start(
                                    wo_t[:], woT[i * 128:(i + 1) * 128,
                                                 mg * 512:(mg + 1) * 512])
                                for j in range(4):
                                    nc.tensor.matmul(
                                        px[j][:],
                                        wo_t[:, j * 128:(j + 1) * 128],
                                        attnT[:, i, :],
                                        start=(i == 0), stop=(i == NM - 1))
                            for j in range(4):
                                m = mg * 4 + j
                                xo = xs2.tile([128, TOK], F32R, tag="xo")
                                nc.sync.dma_start(
                                    xo[:], xT[m * 128:(m + 1) * 128, 512:1024])
                                nc.vector.tensor_add(x2T[:, m, :], px[j][:],
                                                     xo[:])
                                sq2 = xs2.tile([128, TOK], BF16, tag="sq2")
                                nc.vector.tensor_mul(sq2[:], x2T[:, m, :],
                                                     x2T[:, m, :])
                                nc.tensor.matmul(acc2[:], oneb[:], sq2[:],
                                                 start=(m == 0),
                                                 stop=(m == NM - 1))

                        # ln2 -> h2 -> AG inputs (first, so AG starts early)
                        s2b = row_rsqrt_bcast([(acc2, 0, 512)], TOK, HID,
                                                "l2")[0][0]
                        for m in range(NM):
                            h2t = xs2.tile([128, TOK], BF16, tag="h2t")
                            nc.vector.tensor_mul(h2t[:], x2T[:, m, :], s2b[:])
                            for q in range(NQ):
                                eng = nc.sync if q < 2 else nc.scalar
                                eng.dma_start(
                                    ag_in[q][m * 128:(m + 1) * 128, :],
                                    h2t[:, q * QW:(q + 1) * QW])

                        for q in range(NQ):
                            nc.gpsimd.collective_compute(
                                "AllGather", mybir.AluOpType.bypass,
                                replica_groups=[list(range(NCORES))],
                                ins=[ag_in[q].opt()], outs=[ag_out[q].opt()],
                            )

                        # x2 token-major -> DRAM (overlaps the AllGather)
                        for tt in range(4):
                            for grp in range(4):
                                ts = xs2.tile([128, 512], BF16, tag="x2t")
                                for j in range(4):
                                    m = grp * 4 + j
                                    pt = bps.tile([128, 128], BF16, tag="big")
                                    nc.tensor.transpose(
                                        pt[:],
                                        x2T[:, m, tt * 128:(tt + 1) * 128],
                                        identB[:])
                                    nc.vector.tensor_copy(
                                        ts[:, j * 128:(j + 1) * 128], pt[:])
                                nc.gpsimd.dma_start(
                                    out[tt * 128:(tt + 1) * 128,
                                        grp * 512:(grp + 1) * 512], ts[:])

            # ============ TP MLP: chunks of 512 tokens (4 cores x 128 cols)
            with tc.tile_pool(name="gsp", bufs=1) as gsp, \
                 tc.tile_pool(name="mh", bufs=2) as mh, \
                 tc.tile_pool(name="silp", bufs=2) as silp, \
                 tc.tile_pool(name="w2s", bufs=2) as w2sp, \
                 tc.tile_pool(name="po", bufs=8) as pop, \
                 tc.tile_pool(name="mps", bufs=4, space="PSUM") as mps:
                gs = gsp.tile([128, NMI, NCORES * TOK], BF16)
                for q in range(NQ):
                    for quad in range(2):
                        h2c = mh.tile([128, NM, 512], BF16, tag="h2")
                        for i in range(NM):
                            src = ag_out[q][quad * 4:(quad + 1) * 4,
                                            i * 128:(i + 1) * 128, :]
                            nc.sync.dma_start(
                                h2c[:, i, :].rearrange(
                                    "p (c w) -> p c w", c=4),
                                src.rearrange("c p w -> p c w"))
                        for m in range(NMI):
                            a = mps.tile([128, 512], F32, tag="m")
                            for i in range(NM):
                                nc.tensor.matmul(
                                    a[:], w1s[:, i, m * 128:(m + 1) * 128],
                                    h2c[:, i, :],
                                    start=(i == 0), stop=(i == NM - 1))
                            sil = silp.tile([128, 512], BF16, tag="sil")
                            nc.scalar.activation(out=sil[:], in_=a[:],
                                                 func=AF.Silu)
                            b = mps.tile([128, 512], F32, tag="m")
                            for i in range(NM):
                                nc.tensor.matmul(
                                    b[:], w3s[:, i, m * 128:(m + 1) * 128],
                                    h2c[:, i, :],
                                    start=(i == 0), stop=(i == NM - 1))
                            dst = gs[:, m,
                                     quad * 4 * TOK:(quad * 4 + 4) * TOK]
                            dst3 = dst.rearrange("p (c w) -> p c w", c=4)
                            nc.vector.tensor_mul(
                                dst3[:, :, q * QW:(q + 1) * QW],
                                sil[:].rearrange("p (c w) -> p c w", c=4),
                                b[:].rearrange("p (c w) -> p c w", c=4))

                # w2 by output-column slice; pipelined ReduceScatter + tail
                w2tiles = {}

                def load_w2(hs):
                    t = w2sp.tile([128, NMI, 512], BF16, tag="w2c",
                                  name=f"w2c{hs}")
                    for m in range(NMI):
                        nc.gpsimd.dma_start(
                            t[:, m, :], w2T[m * 128:(m + 1) * 128,
                                            hs * 512:(hs + 1) * 512])
                    w2tiles[hs] = t

                load_w2(0)
                load_w2(1)
                for hs in range(4):
                    w2c = w2tiles[hs]
                    for tt in range(NCORES * TOK // 128):
                        op_ = mps.tile([128, 512], F32, tag="m")
                        for m in range(NMI):
                            nc.tensor.matmul(
                                op_[:], gs[:, m, tt * 128:(tt + 1) * 128],
                                w2c[:, m, :],
                                start=(m == 0), stop=(m == NMI - 1))
                        pb = pop.tile([128, 512], BF16, tag="pb")
                        nc.vector.tensor_copy(pb[:], op_[:])
                        nc.sync.dma_start(
                            rs_in[hs][tt * 128:(tt + 1) * 128, :], pb[:])
                    if hs + 2 < 4:
                        load_w2(hs + 2)
                    nc.gpsimd.collective_compute(
                        "ReduceScatter", mybir.AluOpType.add,
                        replica_groups=[list(range(NCORES))],
                        ins=[rs_in[hs].opt()], outs=[rs_out[hs].opt()],
                    )

                for hs in range(4):
                    for tt in range(4):
                        nc.gpsimd.dma_start(
                            out[tt * 128:(tt + 1) * 128,
                                hs * 512:(hs + 1) * 512],
                            rs_out[hs][tt * 128:(tt + 1) * 128, :],
                            accum_op=mybir.AluOpType.add)

    return nc


# ---------------------------------------------------------------- host side
def _rope_tables(pos):
    inv = 1.0 / (THETA ** (np.arange(0, RD, 2, dtype=np.float32) / RD))
    f = pos[:, None].astype(np.float32) * inv[None, :]
    emb = np.concatenate([f, f], axis=-1)          # [T, RD]
    cos = np.ascontiguousarray(np.cos(emb).T)      # [RD, T]
    sin = np.sin(emb).T
    sinS = sin.copy()
    sinS[0:32] = -sin[0:32]
    return cos.astype(np.float32), np.ascontiguousarray(sinS).astype(np.float32)


def _band_mask():
    import ml_dtypes
    p = np.arange(128)[:, None]
    u = np.arange(1408)[None, :]
    m = ((u >= p + 384) & (u <= p + 896)).astype(np.float32)
    return m.astype(ml_dtypes.bfloat16)


def _prepare_in_maps(hidden_states, wq, wk, wv, wo, q_norm_w, k_norm_w,
                     ln1_w, ln2_w, w1, w2, w3):
    import ml_dtypes
    xf = np.ascontiguousarray(hidden_states.reshape(B * S, HID))
    # fold ln1_w into wq/wk/wv rows, ln2_w into w1/w3 rows
    wqTn = np.ascontiguousarray(wq.T * ln1_w[:, None])
    wkTn = np.ascontiguousarray(wk.T * ln1_w[:, None])
    wvTn = np.ascontiguousarray(wv.T * ln1_w[:, None])
    woTn = np.ascontiguousarray(wo.T)
    w1Tn = np.ascontiguousarray(w1.T * ln2_w[:, None])
    w3Tn = np.ascontiguousarray(w3.T * ln2_w[:, None])
    w2Tn = np.ascontiguousarray(w2.T)
    qnc = np.ascontiguousarray(q_norm_w.reshape(16, 128).T)
    knc = np.ascontiguousarray(k_norm_w.reshape(4, 128).T)
    band = _band_mask()
    ones_r = np.ones((1, 128), np.float32)
    ones_b = np.ones((128, 1), ml_dtypes.bfloat16)

    in_maps = []
    for c in range(NCORES):
        t0 = c * TOK
        bidx = t0 // S
        s0 = t0 % S
        xe = np.zeros((EXT, HID), np.float32)
        lo = s0 - WIN
        if lo >= 0:
            xe[:] = xf[bidx * S + lo: bidx * S + s0 + TOK]
            halo_valid = True
        else:
            xe[WIN:] = xf[bidx * S + s0: bidx * S + s0 + TOK]
            halo_valid = False
        xTc = np.ascontiguousarray(xe.T)

        qpos = np.arange(s0, s0 + TOK)
        kpos = np.arange(s0 - WIN, s0 + TOK)
        cq, sq_ = _rope_tables(qpos)
        ck, sk_ = _rope_tables(np.maximum(kpos, 0))
        halo_bias = np.zeros(EXT, np.float32)
        if not halo_valid:
            halo_bias[0:WIN] = NEG
        haloc = np.ascontiguousarray(halo_bias.reshape(8, 128).T)

        in_maps.append({
            "xT": xTc,
            "wqT": wqTn, "wkT": wkTn, "wvT": wvTn, "woT": woTn,
            "w1T": np.ascontiguousarray(w1Tn[:, c * IMC:(c + 1) * IMC]),
            "w3T": np.ascontiguousarray(w3Tn[:, c * IMC:(c + 1) * IMC]),
            "w2T": np.ascontiguousarray(w2Tn[c * IMC:(c + 1) * IMC, :]),
            "qnw": qnc, "knw": knc,
            "cos_q": cq, "sinS_q": sq_, "cos_k": ck, "sinS_k": sk_,
            "halo": haloc, "band": band,
            "ones_r": ones_r, "ones_b": ones_b,
        })
    return in_maps


_NC = None


def _get_nc():
    global _NC
    if _NC is None:
        _register_ntff_hook()
        _NC = build_nc()
    return _NC


def run(in_maps, trace=False):
    from concourse.bass_utils import run_bass_kernel_spmd
    nc = _get_nc()
    return run_bass_kernel_spmd(nc, in_maps, core_ids=list(range(NCORES)),
                                trace=trace)


def kernel(**inputs):
    arrs = {k: np.asarray(v, dtype=np.float32) for k, v in inputs.items()}
    in_maps = _prepare_in_maps(
        arrs["hidden_states"], arrs["wq"], arrs["wk"], arrs["wv"], arrs["wo"],
        arrs["q_norm_w"], arrs["k_norm_w"], arrs["ln1_w"], arrs["ln2_w"],
        arrs["w1"], arrs["w2"], arrs["w3"])
    res = run(in_maps, trace=False)
    full = np.empty((B * S, HID), np.float32)
    for c in range(NCORES):
        full[c * TOK:(c + 1) * TOK] = res.results[c]["out"]
    return full.reshape(B, S, HID)
